# revision 10
# baseline (speedup 1.0000x reference)
"""Trainium2 Bass kernel for a single-step attention LSTM decoder (DecoderRNN).

Strategy (8 NeuronCores, SPMD):
  - The dominant cost is the vocab projection out_W (50257x1024 fp32, ~206 MB):
    sharded over vocab across the 8 cores (~26 MB/core), streamed k-chunk by
    k-chunk through the PE as stationary [128,128] fp32 tiles.
  - LSTM weights W_ih/W_hh (2x 4096x1024) are sharded over the contraction dim:
    each core computes partial pre-activation gates [1,4096] from its 128-wide
    slice of the hidden dim; a 16 KB AllReduce(add) produces full gates on all
    cores.
  - The a2d projection is likewise contraction-sharded for the LSTM input: each
    core only computes its own 128-slice of the a2d output (relu'd), which is
    exactly the slice its W_ih partial product needs.  No extra collective.
  - Attention (tiny) is computed replicated on every core.
  - log_softmax over the vocab: each core computes local (max, sum-exp) over its
    shard, a 64 B AllGather exchanges the 8 stat pairs, each core combines them
    into the global logsumexp and writes its normalized shard.
  - Embedding row gather + weight re-layout (transposes) happen on host inside
    kernel(); the device only streams what the math needs.
"""

import numpy as np

import concourse.bacc as bacc
import concourse.mybir as mybir
from concourse import bass_isa, tile
from concourse.bass_utils import run_bass_kernel_spmd

F32 = mybir.dt.float32
N_CORES = 8
V, H, E, L = 50257, 1024, 512, 128
KC_A = (E + H) // 128          # 12 contraction chunks for attention / a2d
NCH = (V + 127) // 128         # 393 global vocab chunks
SLOTS = (NCH + N_CORES - 1) // N_CORES  # 50 vocab chunks per core
WG = 5                         # vocab slots per streamed weight tile
NEG = -1e30

_CACHE = {}


def _build():
    """Build + compile the SPMD Bass program (same on all cores)."""
    nc = bacc.Bacc(None, num_devices=N_CORES)

    din = {}
    for name, shape in [
        ("attn_in", [128, KC_A]),       # [embed;h0] column-chunk layout
        ("attn_wt", [128, KC_A, 128]),  # attn_W.T re-laid [p, kc, l]
        ("attn_b", [1, 128]),
        ("enc", [L, H]),                # natural layout
        ("a2d_wt", [128, KC_A, 128]),   # per-core slice of a2d_W.T
        ("a2d_b", [128, 1]),            # per-core slice
        ("wih_t", [128, 4 * H]),        # per-core k-slice, gate-interleaved cols
        ("whh_t", [128, 4 * H]),
        ("h0_col", [128, 1]),           # per-core k-slice of h0
        ("b_gates", [128, 32]),         # gate-interleaved bias, natural [128,32]
        ("c0_rm", [128, 8]),            # c0 row-major-p layout
        ("w_out", [SLOTS // WG, 128, WG * H]),  # per-core out_W.T slot-group tiles
        ("b_out", [128, SLOTS]),
    ]:
        din[name] = nc.dram_tensor(name, shape, F32, kind="ExternalInput")

    z_out = nc.dram_tensor("z_out", [128, SLOTS], F32, kind="ExternalOutput")
    h_out = nc.dram_tensor("h_out", [128, 8], F32, kind="ExternalOutput")
    c_out = nc.dram_tensor("c_out", [128, 8], F32, kind="ExternalOutput")
    aw_out = nc.dram_tensor("aw_out", [1, 128], F32, kind="ExternalOutput")

    rg = [list(range(N_CORES))]

    with tile.TileContext(nc) as tc:
        with (
            tc.tile_pool(name="const", bufs=1) as cp,
            tc.tile_pool(name="work", bufs=1) as wp,
            tc.tile_pool(name="wout", bufs=3) as wop,
            tc.tile_pool(name="psum", bufs=1, space="PSUM") as pp,
            tc.tile_pool(name="psmall", bufs=2, space="PSUM") as pps,
            tc.tile_pool(name="dram", bufs=1, space="DRAM") as dp,
        ):
            # ---- front weight loads (scalar HWDGE ring, before the big stream)
            attn_wt_sb = cp.tile([128, KC_A, 128], F32, tag="attn_wt")
            nc.scalar.dma_start(attn_wt_sb[:], din["attn_wt"][:])
            a2d_wt_sb = cp.tile([128, KC_A, 128], F32, tag="a2d_wt")
            nc.scalar.dma_start(a2d_wt_sb[:], din["a2d_wt"][:])
            enc_sb = cp.tile([L, H], F32, tag="enc")
            nc.scalar.dma_start(enc_sb[:], din["enc"][:])
            wih_sb = cp.tile([128, 4 * H], F32, tag="wih")
            nc.scalar.dma_start(wih_sb[:], din["wih_t"][:])
            whh_sb = cp.tile([128, 4 * H], F32, tag="whh")
            nc.scalar.dma_start(whh_sb[:], din["whh_t"][:])

            # ---- small loads (sync ring)
            attn_in_sb = cp.tile([128, KC_A], F32, tag="attn_in")
            nc.sync.dma_start(attn_in_sb[:], din["attn_in"][:])
            attn_b_sb = cp.tile([1, 128], F32, tag="attn_b")
            nc.sync.dma_start(attn_b_sb[:], din["attn_b"][:])
            a2d_b_sb = cp.tile([128, 1], F32, tag="a2d_b")
            nc.sync.dma_start(a2d_b_sb[:], din["a2d_b"][:])
            h0c_sb = cp.tile([128, 1], F32, tag="h0c")
            nc.sync.dma_start(h0c_sb[:], din["h0_col"][:])
            bg_sb = cp.tile([128, 32], F32, tag="bg")
            nc.sync.dma_start(bg_sb[:], din["b_gates"][:])
            c0_sb = cp.tile([128, 8], F32, tag="c0")
            nc.sync.dma_start(c0_sb[:], din["c0_rm"][:])
            bout_sb = cp.tile([128, SLOTS], F32, tag="bout")
            nc.sync.dma_start(bout_sb[:], din["b_out"][:])

            ones11 = cp.tile([1, 1], F32, tag="ones11")
            nc.vector.memset(ones11[:], 1.0)
            ones_row = cp.tile([1, 128], F32, tag="ones_row")
            nc.vector.memset(ones_row[:], 1.0)

            # ================= attention (replicated) =================
            ps_a = pps.tile([1, 128], F32, tag="small")
            for kc in range(KC_A):
                nc.tensor.matmul(
                    ps_a[:], attn_in_sb[:, kc : kc + 1], attn_wt_sb[:, kc, :],
                    start=(kc == 0), stop=(kc == KC_A - 1),
                )
            za = wp.tile([1, 128], F32, tag="za")
            nc.vector.tensor_add(za[:], ps_a[:], attn_b_sb[:])
            # softmax over the 128 logits (all on partition 0)
            mneg = wp.tile([1, 1], F32, tag="mneg")
            nc.vector.reduce_max(mneg[:], za[:], axis=mybir.AxisListType.X, negate=True)
            ew = wp.tile([1, 128], F32, tag="ew")
            ssum = wp.tile([1, 1], F32, tag="ssum")
            nc.scalar.activation(
                ew[:], za[:], mybir.ActivationFunctionType.Exp,
                bias=mneg[:], accum_out=ssum[:],
            )
            rsum = wp.tile([1, 1], F32, tag="rsum")
            nc.vector.reciprocal(rsum[:], ssum[:])
            w_row = wp.tile([1, 128], F32, tag="w_row")
            nc.vector.tensor_scalar_mul(w_row[:], ew[:], rsum[:])
            nc.sync.dma_start(aw_out[:], w_row[:])

            # transpose attn weights to a column: [1,128] -> [128,1]
            ps_w = pps.tile([128, 1], F32, tag="small")
            nc.tensor.matmul(ps_w[:], w_row[:], ones11[:], start=True, stop=True)
            wcol = wp.tile([128, 1], F32, tag="wcol")
            nc.vector.tensor_copy(wcol[:], ps_w[:])

            # attn_out = attn_weight @ enc_output, in column-chunk layout [128,8]
            ps_c = pps.tile([128, 8], F32, tag="small")
            for j in range(8):
                nc.tensor.matmul(
                    ps_c[:, j : j + 1], enc_sb[:, j * 128 : (j + 1) * 128], wcol[:],
                    start=True, stop=True,
                )
            dec_in = wp.tile([128, KC_A], F32, tag="dec_in")
            nc.vector.tensor_copy(dec_in[:, 0:4], attn_in_sb[:, 0:4])
            nc.vector.tensor_copy(dec_in[:, 4:12], ps_c[:])

            # ========== a2d: only this core's 128-slice of the output ==========
            ps_x = pps.tile([128, 1], F32, tag="small")
            for kc in range(KC_A):
                nc.tensor.matmul(
                    ps_x[:], a2d_wt_sb[:, kc, :], dec_in[:, kc : kc + 1],
                    start=(kc == 0), stop=(kc == KC_A - 1),
                )
            x_col = wp.tile([128, 1], F32, tag="x_col")
            nc.scalar.activation(
                x_col[:], ps_x[:], mybir.ActivationFunctionType.Relu, bias=a2d_b_sb[:]
            )

            # ========== LSTM gates: partial [1, 4096], AllReduce ==========
            g_dram = dp.tile([1, 4 * H], F32)
            g_part = wp.tile([1, 4 * H], F32, tag="g_part")
            for half in range(2):
                ps_g = pp.tile([1, 2 * H], F32, tag="gates")
                for b in range(4):
                    lo = b * 512
                    nc.tensor.matmul(
                        ps_g[:, lo : lo + 512],
                        x_col[:], wih_sb[:, half * 2048 + lo : half * 2048 + lo + 512],
                        start=True, stop=False,
                    )
                    nc.tensor.matmul(
                        ps_g[:, lo : lo + 512],
                        h0c_sb[:], whh_sb[:, half * 2048 + lo : half * 2048 + lo + 512],
                        start=False, stop=True,
                    )
                for b in range(4):
                    # PSUM can't be DMA'd; copy banks out on alternating engines
                    eng = nc.vector if b % 2 == 0 else nc.scalar
                    dst = g_part[:, half * 2048 + b * 512 : half * 2048 + (b + 1) * 512]
                    if eng is nc.vector:
                        eng.tensor_copy(dst, ps_g[:, b * 512 : (b + 1) * 512])
                    else:
                        eng.activation(
                            dst, ps_g[:, b * 512 : (b + 1) * 512],
                            mybir.ActivationFunctionType.Copy,
                        )
            nc.sync.dma_start(g_dram[:], g_part[:])
            g_all = dp.tile([1, 4 * H], F32)
            nc.gpsimd.collective_compute(
                "AllReduce", mybir.AluOpType.add, replica_groups=rg,
                ins=[g_dram.opt()], outs=[g_all.opt()],
            )

            # gates -> [128, 32] natural (row-major-p, gate-type interleaved)
            g_sb = wp.tile([128, 32], F32, tag="g_sb")
            nc.sync.dma_start(g_sb[:], g_all[:].rearrange("a (p f) -> (a p) f", p=128))
            t1 = wp.tile([128, 32], F32, tag="t1")
            nc.vector.tensor_add(t1[:], g_sb[:], bg_sb[:])
            t1v = t1[:].rearrange("p (a b) -> p a b", b=4)

            AF = mybir.ActivationFunctionType
            sig_i = wp.tile([128, 8], F32, tag="sig_i")
            nc.scalar.activation(sig_i[:], t1v[:, :, 0], AF.Sigmoid)
            sig_f = wp.tile([128, 8], F32, tag="sig_f")
            nc.scalar.activation(sig_f[:], t1v[:, :, 1], AF.Sigmoid)
            tanh_g = wp.tile([128, 8], F32, tag="tanh_g")
            nc.scalar.activation(tanh_g[:], t1v[:, :, 2], AF.Tanh)
            sig_o = wp.tile([128, 8], F32, tag="sig_o")
            nc.scalar.activation(sig_o[:], t1v[:, :, 3], AF.Sigmoid)

            c_new = wp.tile([128, 8], F32, tag="c_new")
            tmp = wp.tile([128, 8], F32, tag="tmp")
            nc.vector.tensor_mul(tmp[:], sig_f[:], c0_sb[:])
            nc.vector.tensor_mul(c_new[:], sig_i[:], tanh_g[:])
            nc.vector.tensor_add(c_new[:], c_new[:], tmp[:])
            tanh_c = wp.tile([128, 8], F32, tag="tanh_c")
            nc.scalar.activation(tanh_c[:], c_new[:], AF.Tanh)
            h_new = wp.tile([128, 8], F32, tag="h_new")
            nc.vector.tensor_mul(h_new[:], sig_o[:], tanh_c[:])
            nc.sync.dma_start(h_out[:], h_new[:])
            nc.sync.dma_start(c_out[:], c_new[:])

            # ========== vocab projection: this core's 50 chunks ==========
            # Slot-major: each slot's 8-chunk accumulation group completes
            # before the next group's start=True clears the PSUM bank state.
            ps_z = pp.tile([128, SLOTS], F32, tag="z")
            for sg in range(SLOTS // WG):
                w_sb = wop.tile([128, WG * H], F32, tag="w")
                nc.scalar.dma_start(w_sb[:], din["w_out"][sg])
                for g in range(WG):
                    s = sg * WG + g
                    for kc in range(8):
                        nc.tensor.matmul(
                            ps_z[:, s : s + 1],
                            w_sb[:, g * H + kc * 128 : g * H + (kc + 1) * 128],
                            h_new[:, kc : kc + 1],
                            start=(kc == 0), stop=(kc == 7),
                        )

            z_sb = wp.tile([128, SLOTS], F32, tag="z_sb")
            nc.vector.tensor_add(z_sb[:], ps_z[:], bout_sb[:])

            # local log-softmax stats over the shard
            mrow = wp.tile([128, 1], F32, tag="mrow")
            nc.vector.reduce_max(mrow[:], z_sb[:], axis=mybir.AxisListType.X)
            mall = wp.tile([128, 1], F32, tag="mall")
            nc.gpsimd.partition_all_reduce(
                mall[:], mrow[:], channels=128, reduce_op=bass_isa.ReduceOp.max
            )
            negm = wp.tile([128, 1], F32, tag="negm")
            nc.vector.tensor_scalar_mul(negm[:], mall[:], -1.0)
            ez = wp.tile([128, SLOTS], F32, tag="ez")
            srow = wp.tile([128, 1], F32, tag="srow")
            nc.scalar.activation(
                ez[:], z_sb[:], AF.Exp, bias=negm[:], accum_out=srow[:]
            )
            sall = wp.tile([128, 1], F32, tag="sall")
            nc.gpsimd.partition_all_reduce(
                sall[:], srow[:], channels=128, reduce_op=bass_isa.ReduceOp.add
            )
            stats = wp.tile([1, 2], F32, tag="stats")
            nc.vector.tensor_copy(stats[:, 0:1], mall[0:1, :])
            nc.vector.tensor_copy(stats[:, 1:2], sall[0:1, :])

            st_in = dp.tile([1, 2], F32)
            st_all = dp.tile([N_CORES, 2], F32)
            nc.sync.dma_start(st_in[:], stats[:])
            nc.gpsimd.collective_compute(
                "AllGather", mybir.AluOpType.bypass, replica_groups=rg,
                ins=[st_in.opt()], outs=[st_all.opt()],
            )
            st_sb = wp.tile([1, N_CORES, 2], F32, tag="st_sb")
            nc.sync.dma_start(st_sb[:], st_all[:])
            stv = st_sb[:]

            mg = wp.tile([1, 1], F32, tag="mg")
            nc.vector.reduce_max(mg[:], stv[:, :, 0], axis=mybir.AxisListType.X)
            negmg = wp.tile([1, 1], F32, tag="negmg")
            nc.vector.tensor_scalar_mul(negmg[:], mg[:], -1.0)
            terms = wp.tile([1, 8], F32, tag="terms")
            nc.scalar.activation(terms[:], stv[:, :, 0], AF.Exp, bias=negmg[:])
            nc.vector.tensor_mul(terms[:], terms[:], stv[:, :, 1])
            stot = wp.tile([1, 1], F32, tag="stot")
            nc.vector.reduce_sum(stot[:], terms[:], axis=mybir.AxisListType.X)
            lnst = wp.tile([1, 1], F32, tag="lnst")
            nc.scalar.activation(lnst[:], stot[:], AF.Ln)
            neglse = wp.tile([1, 1], F32, tag="neglse")
            nc.vector.tensor_add(neglse[:], lnst[:], mg[:])
            nc.vector.tensor_scalar_mul(neglse[:], neglse[:], -1.0)

            ps_b = pps.tile([128, 1], F32, tag="small")
            nc.tensor.matmul(ps_b[:], ones_row[:], neglse[:], start=True, stop=True)
            negl_col = wp.tile([128, 1], F32, tag="negl_col")
            nc.vector.tensor_copy(negl_col[:], ps_b[:])

            zfin = wp.tile([128, SLOTS], F32, tag="zfin")
            nc.vector.tensor_scalar_add(zfin[:], z_sb[:], negl_col[:])
            nc.sync.dma_start(z_out[:], zfin[:])

    nc.compile()
    return nc


def _prep_inputs(input_tok, h0, c0, enc_output, emb_table, attn_W, attn_b,
                 a2d_W, a2d_b, W_ih, W_hh, b_ih, b_hh, out_W, out_b):
    """Host-side sharding / re-layout. Returns per-core input maps + chunk map."""
    f32 = np.float32
    tok = int(np.asarray(input_tok).reshape(-1)[0])
    embed = np.asarray(emb_table[tok], dtype=f32).reshape(E)
    h0v = np.asarray(h0, dtype=f32).reshape(H)
    c0v = np.asarray(c0, dtype=f32).reshape(H)
    enc = np.ascontiguousarray(np.asarray(enc_output, dtype=f32))

    attn_in = np.concatenate([embed, h0v]).reshape(KC_A, 128).T.copy()  # [128,12]
    attn_wt = (
        np.asarray(attn_W, dtype=f32).T.reshape(KC_A, 128, 128).transpose(1, 0, 2).copy()
    )  # [128, kc, l]
    attn_b_r = np.asarray(attn_b, dtype=f32).reshape(1, 128).copy()

    a2d_wt_full = np.asarray(a2d_W, dtype=f32).T  # (1536, 1024)
    b_gate = (np.asarray(b_ih, dtype=f32) + np.asarray(b_hh, dtype=f32))
    b_gate_il = b_gate.reshape(4, H).T.reshape(128, 32).copy()  # interleaved
    c0_rm = c0v.reshape(128, 8).copy()

    # gate-interleaved, transposed LSTM weights (1024, 4096)
    wih_il_t = np.asarray(W_ih, dtype=f32).reshape(4, H, H).transpose(2, 1, 0).reshape(H, 4 * H)
    whh_il_t = np.asarray(W_hh, dtype=f32).reshape(4, H, H).transpose(2, 1, 0).reshape(H, 4 * H)

    out_Wf = np.asarray(out_W, dtype=f32)
    out_bf = np.asarray(out_b, dtype=f32)

    chunk_map = [[] for _ in range(N_CORES)]
    for j in range(NCH):
        chunk_map[j % N_CORES].append(j)
    for c in range(N_CORES):
        while len(chunk_map[c]) < SLOTS:
            chunk_map[c].append(-1)  # zero-pad slot

    in_maps = []
    for c in range(N_CORES):
        chunks = chunk_map[c]
        arr = np.zeros((SLOTS, 128, H), dtype=f32)
        b_out_c = np.full((128, SLOTS), NEG, dtype=f32)
        for s, j in enumerate(chunks):
            if j < 0:
                continue
            lo, hi = j * 128, min((j + 1) * 128, V)
            n = hi - lo
            arr[s, :n] = out_Wf[lo:hi]
            b_out_c[:n, s] = out_bf[lo:hi]
        # arr[s, i, r] with r = p*8 + kc  ->  tile[sg, p, g*H + kc*128 + i]
        w_out_c = np.ascontiguousarray(
            arr.reshape(SLOTS, 128, 128, 8)   # (s, i, p, kc)
            .transpose(0, 2, 3, 1)            # (s, p, kc, i)
            .reshape(SLOTS // WG, WG, 128, H) # (sg, g, p, kc*128+i)
            .transpose(0, 2, 1, 3)            # (sg, p, g, f)
            .reshape(SLOTS // WG, 128, WG * H)
        )
        a2d_wt_c = np.ascontiguousarray(
            a2d_wt_full[:, c * 128 : (c + 1) * 128]
            .reshape(KC_A, 128, 128)
            .transpose(1, 0, 2)
        )
        in_maps.append({
            "attn_in": attn_in,
            "attn_wt": attn_wt,
            "attn_b": attn_b_r,
            "enc": enc,
            "a2d_wt": a2d_wt_c,
            "a2d_b": np.asarray(a2d_b, dtype=f32)[c * 128 : (c + 1) * 128].reshape(128, 1).copy(),
            "wih_t": np.ascontiguousarray(wih_il_t[c * 128 : (c + 1) * 128]),
            "whh_t": np.ascontiguousarray(whh_il_t[c * 128 : (c + 1) * 128]),
            "h0_col": h0v[c * 128 : (c + 1) * 128].reshape(128, 1).copy(),
            "b_gates": b_gate_il,
            "c0_rm": c0_rm,
            "w_out": w_out_c,
            "b_out": np.ascontiguousarray(b_out_c),
        })
    return in_maps, chunk_map


def kernel(**inputs):
    outputs, _ = _run(inputs, trace=False)
    return outputs


def run_traced(inputs):
    """test-only entry: returns (outputs, BassKernelResults) with a HW trace."""
    return _run(inputs, trace=True)


def _run(inputs, trace):
    if "nc" not in _CACHE:
        _CACHE["nc"] = _build()
    nc = _CACHE["nc"]

    in_maps, chunk_map = _prep_inputs(**inputs)
    res = run_bass_kernel_spmd(
        nc, in_maps, core_ids=list(range(N_CORES)), trace=trace
    )
    results = res.results

    full = np.empty(NCH * 128, dtype=np.float32)
    for c in range(N_CORES):
        z_c = results[c]["z_out"]  # (128, SLOTS)
        for s, j in enumerate(chunk_map[c]):
            if j >= 0:
                full[j * 128 : (j + 1) * 128] = z_c[:, s]
    output = full[:V].reshape(1, V)

    h_new = results[0]["h_out"].reshape(1, 1, H).astype(np.float32)
    c_new = results[0]["c_out"].reshape(1, 1, H).astype(np.float32)
    attn_w = results[0]["aw_out"].reshape(1, L).astype(np.float32)
    return (output, h_new, c_new, attn_w), res


# revision 13
# speedup vs baseline: 1.5183x; 1.5183x over previous
"""Trainium2 Bass kernel for a single-step attention LSTM decoder (DecoderRNN).

Strategy (8 NeuronCores, SPMD):
  - The dominant cost is the vocab projection out_W (50257x1024 fp32, ~206 MB):
    sharded over vocab across the 8 cores (~26 MB/core), streamed in 8 waves of
    [contraction=1024, vocab=800] fp32r tiles used as the PE's moving operand
    (h stationary) -- ~45us of PE time, fully hidden under the DMA stream.
  - LSTM weights W_ih/W_hh (2x 4096x1024) are sharded over the contraction dim:
    each core computes partial pre-activation gates [1,4096] from its 128-wide
    slice of the hidden dim; a 16 KB AllReduce(add) produces full gates on all
    cores.
  - The a2d projection is likewise contraction-sharded for the LSTM input: each
    core only computes its own 128-slice of the a2d output (relu'd), which is
    exactly the slice its W_ih partial product needs.  No extra collective.
  - Attention (tiny) is computed replicated on every core.
  - log_softmax over the vocab: per-wave (max, sum-exp) stats on partition 0,
    combined locally, then a 64 B AllGather exchanges the 8 core-local stat
    pairs; each core computes the global logsumexp and normalizes its shard.
  - Embedding row gather + weight re-layout (transposes) happen on host inside
    kernel(); the device only streams what the math needs.
"""

import numpy as np

import concourse.bacc as bacc
import concourse.mybir as mybir
from concourse import tile
from concourse.bass_utils import run_bass_kernel_spmd

F32 = mybir.dt.float32
F32R = mybir.dt.float32r
AF = mybir.ActivationFunctionType
AX = mybir.AxisListType
N_CORES = 8
V, H, E, L = 50257, 1024, 512, 128
KC_A = (E + H) // 128          # 12 contraction chunks for attention / a2d
NCH = (V + 127) // 128         # 393 global vocab chunks
SLOTS = (NCH + N_CORES - 1) // N_CORES  # 50 vocab chunks per core
VCORE = SLOTS * 128            # 6400 vocab columns per core
NW, WM = 8, 800                # weight stream: 8 waves x 800 vocab cols
NEG = -1e30

_CACHE = {}


def _build():
    """Build + compile the SPMD Bass program (same on all cores)."""
    nc = bacc.Bacc(None, num_devices=N_CORES)

    din = {}
    for name, shape in [
        ("attn_in", [128, KC_A]),       # [embed;h0] column-chunk layout
        ("attn_wt", [128, KC_A, 128]),  # attn_W.T re-laid [p, kc, l]
        ("attn_b", [1, 128]),
        ("enc", [L, H]),                # natural layout
        ("a2d_wt", [128, KC_A, 128]),   # per-core slice of a2d_W.T
        ("a2d_b", [128, 1]),            # per-core slice
        ("wih_t", [128, 4 * H]),        # per-core k-slice, gate-interleaved cols
        ("whh_t", [128, 4 * H]),
        ("h0_col", [128, 1]),           # per-core k-slice of h0
        ("b_gates", [128, 32]),         # gate-interleaved bias, natural [128,32]
        ("c0_rm", [128, 8]),            # c0 row-major-p layout
        ("b_row", [1, VCORE]),          # out_b shard (padded with -1e30)
    ]:
        din[name] = nc.dram_tensor(name, shape, F32, kind="ExternalInput")
    din["w_out"] = nc.dram_tensor("w_out", [NW, 128, 8, WM], F32R,
                                  kind="ExternalInput")

    z_out = nc.dram_tensor("z_out", [1, VCORE], F32, kind="ExternalOutput")
    h_out = nc.dram_tensor("h_out", [128, 8], F32, kind="ExternalOutput")
    c_out = nc.dram_tensor("c_out", [128, 8], F32, kind="ExternalOutput")
    aw_out = nc.dram_tensor("aw_out", [1, 128], F32, kind="ExternalOutput")

    rg = [list(range(N_CORES))]

    with tile.TileContext(nc) as tc:
        with (
            tc.tile_pool(name="const", bufs=1) as cp,
            tc.tile_pool(name="work", bufs=1) as wp,
            tc.tile_pool(name="wave", bufs=3) as wvp,
            tc.tile_pool(name="dram", bufs=1, space="DRAM") as dp,
        ):
            # ---- front weight loads (scalar HWDGE ring, before the big stream)
            attn_wt_sb = cp.tile([128, KC_A, 128], F32, tag="attn_wt")
            nc.scalar.dma_start(attn_wt_sb[:], din["attn_wt"][:])
            a2d_wt_sb = cp.tile([128, KC_A, 128], F32, tag="a2d_wt")
            nc.scalar.dma_start(a2d_wt_sb[:], din["a2d_wt"][:])
            enc_sb = cp.tile([L, H], F32, tag="enc")
            nc.scalar.dma_start(enc_sb[:], din["enc"][:])
            wih_sb = cp.tile([128, 4 * H], F32, tag="wih")
            nc.scalar.dma_start(wih_sb[:], din["wih_t"][:])
            whh_sb = cp.tile([128, 4 * H], F32, tag="whh")
            nc.scalar.dma_start(whh_sb[:], din["whh_t"][:])

            # ---- small loads (sync ring)
            attn_in_sb = cp.tile([128, KC_A], F32, tag="attn_in")
            nc.sync.dma_start(attn_in_sb[:], din["attn_in"][:])
            attn_b_sb = cp.tile([1, 128], F32, tag="attn_b")
            nc.sync.dma_start(attn_b_sb[:], din["attn_b"][:])
            a2d_b_sb = cp.tile([128, 1], F32, tag="a2d_b")
            nc.sync.dma_start(a2d_b_sb[:], din["a2d_b"][:])
            h0c_sb = cp.tile([128, 1], F32, tag="h0c")
            nc.sync.dma_start(h0c_sb[:], din["h0_col"][:])
            bg_sb = cp.tile([128, 32], F32, tag="bg")
            nc.sync.dma_start(bg_sb[:], din["b_gates"][:])
            c0_sb = cp.tile([128, 8], F32, tag="c0")
            nc.sync.dma_start(c0_sb[:], din["c0_rm"][:])
            brow_sb = cp.tile([1, VCORE], F32, tag="brow")
            nc.sync.dma_start(brow_sb[:], din["b_row"][:])

            ones11 = cp.tile([1, 1], F32, tag="ones11")
            nc.vector.memset(ones11[:], 1.0)

            with (
                tc.tile_pool(name="pfront", bufs=1, space="PSUM") as pp,
                tc.tile_pool(name="psmall", bufs=2, space="PSUM") as pps,
            ):
                # ================= attention (replicated) =================
                ps_a = pps.tile([1, 128], F32, tag="small")
                for kc in range(KC_A):
                    nc.tensor.matmul(
                        ps_a[:], attn_in_sb[:, kc : kc + 1], attn_wt_sb[:, kc, :],
                        start=(kc == 0), stop=(kc == KC_A - 1),
                    )
                za = wp.tile([1, 128], F32, tag="za")
                nc.vector.tensor_add(za[:], ps_a[:], attn_b_sb[:])
                # softmax over the 128 logits (all on partition 0)
                mneg = wp.tile([1, 1], F32, tag="mneg")
                nc.vector.reduce_max(mneg[:], za[:], axis=AX.X, negate=True)
                ew = wp.tile([1, 128], F32, tag="ew")
                ssum = wp.tile([1, 1], F32, tag="ssum")
                nc.scalar.activation(ew[:], za[:], AF.Exp,
                                     bias=mneg[:], accum_out=ssum[:])
                rsum = wp.tile([1, 1], F32, tag="rsum")
                nc.vector.reciprocal(rsum[:], ssum[:])
                w_row = wp.tile([1, 128], F32, tag="w_row")
                nc.vector.tensor_scalar_mul(w_row[:], ew[:], rsum[:])
                nc.sync.dma_start(aw_out[:], w_row[:])

                # transpose attn weights to a column: [1,128] -> [128,1]
                ps_w = pps.tile([128, 1], F32, tag="small")
                nc.tensor.matmul(ps_w[:], w_row[:], ones11[:], start=True, stop=True)
                wcol = wp.tile([128, 1], F32, tag="wcol")
                nc.vector.tensor_copy(wcol[:], ps_w[:])

                # attn_out = attn_weight @ enc_output, column-chunk layout [128,8]
                ps_c = pps.tile([128, 8], F32, tag="small")
                for j in range(8):
                    nc.tensor.matmul(
                        ps_c[:, j : j + 1], enc_sb[:, j * 128 : (j + 1) * 128],
                        wcol[:], start=True, stop=True,
                    )
                dec_in = wp.tile([128, KC_A], F32, tag="dec_in")
                nc.vector.tensor_copy(dec_in[:, 0:4], attn_in_sb[:, 0:4])
                nc.vector.tensor_copy(dec_in[:, 4:12], ps_c[:])

                # ===== a2d: only this core's 128-slice of the output =====
                ps_x = pps.tile([128, 1], F32, tag="small")
                for kc in range(KC_A):
                    nc.tensor.matmul(
                        ps_x[:], a2d_wt_sb[:, kc, :], dec_in[:, kc : kc + 1],
                        start=(kc == 0), stop=(kc == KC_A - 1),
                    )
                x_col = wp.tile([128, 1], F32, tag="x_col")
                nc.scalar.activation(x_col[:], ps_x[:], AF.Relu, bias=a2d_b_sb[:])

                # ===== LSTM gates: partial [1, 4096], AllReduce =====
                g_dram = dp.tile([1, 4 * H], F32)
                g_part = wp.tile([1, 4 * H], F32, tag="g_part")
                for half in range(2):
                    ps_g = pp.tile([1, 2 * H], F32, tag="gates")
                    for b in range(4):
                        lo = b * 512
                        nc.tensor.matmul(
                            ps_g[:, lo : lo + 512], x_col[:],
                            wih_sb[:, half * 2048 + lo : half * 2048 + lo + 512],
                            start=True, stop=False,
                        )
                        nc.tensor.matmul(
                            ps_g[:, lo : lo + 512], h0c_sb[:],
                            whh_sb[:, half * 2048 + lo : half * 2048 + lo + 512],
                            start=False, stop=True,
                        )
                    for b in range(4):
                        # PSUM can't be DMA'd; copy banks out on two engines
                        dst = g_part[:, half * 2048 + b * 512 :
                                     half * 2048 + (b + 1) * 512]
                        if b % 2 == 0:
                            nc.vector.tensor_copy(dst, ps_g[:, b * 512 : (b + 1) * 512])
                        else:
                            nc.scalar.activation(
                                dst, ps_g[:, b * 512 : (b + 1) * 512], AF.Copy)
                nc.sync.dma_start(g_dram[:], g_part[:])
                g_all = dp.tile([1, 4 * H], F32)
                nc.gpsimd.collective_compute(
                    "AllReduce", mybir.AluOpType.add, replica_groups=rg,
                    ins=[g_dram.opt()], outs=[g_all.opt()],
                )

                # gates -> [128, 32] natural (row-major-p, gate-type interleaved)
                g_sb = wp.tile([128, 32], F32, tag="g_sb")
                nc.sync.dma_start(
                    g_sb[:], g_all[:].rearrange("a (p f) -> (a p) f", p=128))
                t1 = wp.tile([128, 32], F32, tag="t1")
                nc.vector.tensor_add(t1[:], g_sb[:], bg_sb[:])
                t1v = t1[:].rearrange("p (a b) -> p a b", b=4)

                sig_i = wp.tile([128, 8], F32, tag="sig_i")
                nc.scalar.activation(sig_i[:], t1v[:, :, 0], AF.Sigmoid)
                sig_f = wp.tile([128, 8], F32, tag="sig_f")
                nc.scalar.activation(sig_f[:], t1v[:, :, 1], AF.Sigmoid)
                tanh_g = wp.tile([128, 8], F32, tag="tanh_g")
                nc.scalar.activation(tanh_g[:], t1v[:, :, 2], AF.Tanh)
                sig_o = wp.tile([128, 8], F32, tag="sig_o")
                nc.scalar.activation(sig_o[:], t1v[:, :, 3], AF.Sigmoid)

                c_new = wp.tile([128, 8], F32, tag="c_new")
                tmp = wp.tile([128, 8], F32, tag="tmp")
                nc.vector.tensor_mul(tmp[:], sig_f[:], c0_sb[:])
                nc.vector.tensor_mul(c_new[:], sig_i[:], tanh_g[:])
                nc.vector.tensor_add(c_new[:], c_new[:], tmp[:])
                tanh_c = wp.tile([128, 8], F32, tag="tanh_c")
                nc.scalar.activation(tanh_c[:], c_new[:], AF.Tanh)
                h_new = wp.tile([128, 8], F32, tag="h_new")
                nc.vector.tensor_mul(h_new[:], sig_o[:], tanh_c[:])
                nc.sync.dma_start(h_out[:], h_new[:])
                nc.sync.dma_start(c_out[:], c_new[:])
                h_r = wp.tile([128, 8], F32R, tag="h_r")
                nc.vector.tensor_copy(h_r[:], h_new[:])

            # ========== vocab projection: 8 waves of [1024 x 800] ==========
            z_row = wp.tile([1, VCORE], F32, tag="z_row")
            negm_all = wp.tile([1, NW], F32, tag="negm_all")
            s_all = wp.tile([1, NW], F32, tag="s_all")
            with tc.tile_pool(name="pz", bufs=2, space="PSUM") as zp:
                for w in range(NW):
                    wv = wvp.tile([128, 8, WM], F32R, tag="wv")
                    nc.scalar.dma_start(wv[:], din["w_out"][w])
                    ps = zp.tile([1, WM], F32, tag="zps")
                    for lo, n in ((0, 512), (512, WM - 512)):
                        for kc in range(8):
                            nc.tensor.matmul(
                                ps[:, lo : lo + n], h_r[:, kc : kc + 1],
                                wv[:, kc, lo : lo + n],
                                start=(kc == 0), stop=(kc == 7),
                            )
                    seg = z_row[:, w * WM : (w + 1) * WM]
                    nc.vector.tensor_add(
                        seg, ps[:], brow_sb[:, w * WM : (w + 1) * WM])
                    nc.vector.reduce_max(
                        negm_all[:, w : w + 1], seg, axis=AX.X, negate=True)
                    e_scr = wp.tile([1, WM], F32, tag="e_scr")
                    nc.scalar.activation(
                        e_scr[:], seg, AF.Exp,
                        bias=negm_all[:, w : w + 1],
                        accum_out=s_all[:, w : w + 1],
                    )

                # local stats:  m_loc = max_w m_w,  s_loc = sum_w s_w*e^(m_w-m_loc)
                negm_loc = wp.tile([1, 1], F32, tag="negm_loc")
                nc.vector.tensor_reduce(
                    negm_loc[:], negm_all[:], axis=AX.X, op=mybir.AluOpType.min)
                terms = wp.tile([1, NW], F32, tag="terms")
                nc.scalar.activation(terms[:], negm_all[:], AF.Exp,
                                     bias=negm_loc[:], scale=-1.0)
                nc.vector.tensor_mul(terms[:], terms[:], s_all[:])
                s_loc = wp.tile([1, 1], F32, tag="s_loc")
                nc.vector.reduce_sum(s_loc[:], terms[:], axis=AX.X)
                stats = wp.tile([1, 2], F32, tag="stats")
                nc.vector.tensor_scalar_mul(stats[:, 0:1], negm_loc[:], -1.0)
                nc.vector.tensor_copy(stats[:, 1:2], s_loc[:])

                st_in = dp.tile([1, 2], F32)
                st_all = dp.tile([N_CORES, 2], F32)
                nc.sync.dma_start(st_in[:], stats[:])
                nc.gpsimd.collective_compute(
                    "AllGather", mybir.AluOpType.bypass, replica_groups=rg,
                    ins=[st_in.opt()], outs=[st_all.opt()],
                )
                st_sb = wp.tile([1, N_CORES, 2], F32, tag="st_sb")
                nc.sync.dma_start(st_sb[:], st_all[:])

                mg = wp.tile([1, 1], F32, tag="mg")
                nc.vector.reduce_max(mg[:], st_sb[:, :, 0], axis=AX.X)
                negmg = wp.tile([1, 1], F32, tag="negmg")
                nc.vector.tensor_scalar_mul(negmg[:], mg[:], -1.0)
                terms2 = wp.tile([1, N_CORES], F32, tag="terms2")
                nc.scalar.activation(terms2[:], st_sb[:, :, 0], AF.Exp,
                                     bias=negmg[:])
                nc.vector.tensor_mul(terms2[:], terms2[:], st_sb[:, :, 1])
                stot = wp.tile([1, 1], F32, tag="stot")
                nc.vector.reduce_sum(stot[:], terms2[:], axis=AX.X)
                lnst = wp.tile([1, 1], F32, tag="lnst")
                nc.scalar.activation(lnst[:], stot[:], AF.Ln)
                neglse = wp.tile([1, 1], F32, tag="neglse")
                nc.vector.tensor_sub(neglse[:], negmg[:], lnst[:])

                # z -= lse, split across DVE and ACT, then store
                half = VCORE // 2
                nc.vector.tensor_scalar_add(
                    z_row[:, 0:half], z_row[:, 0:half], neglse[:])
                nc.scalar.activation(
                    z_row[:, half:VCORE], z_row[:, half:VCORE], AF.Identity,
                    bias=neglse[:])
                nc.sync.dma_start(z_out[:], z_row[:])

    nc.compile()
    return nc


def _prep_inputs(input_tok, h0, c0, enc_output, emb_table, attn_W, attn_b,
                 a2d_W, a2d_b, W_ih, W_hh, b_ih, b_hh, out_W, out_b):
    """Host-side sharding / re-layout. Returns per-core input maps + chunk map."""
    f32 = np.float32
    tok = int(np.asarray(input_tok).reshape(-1)[0])
    embed = np.asarray(emb_table[tok], dtype=f32).reshape(E)
    h0v = np.asarray(h0, dtype=f32).reshape(H)
    c0v = np.asarray(c0, dtype=f32).reshape(H)
    enc = np.ascontiguousarray(np.asarray(enc_output, dtype=f32))

    attn_in = np.concatenate([embed, h0v]).reshape(KC_A, 128).T.copy()  # [128,12]
    attn_wt = (
        np.asarray(attn_W, dtype=f32).T.reshape(KC_A, 128, 128).transpose(1, 0, 2).copy()
    )  # [128, kc, l]
    attn_b_r = np.asarray(attn_b, dtype=f32).reshape(1, 128).copy()

    a2d_wt_full = np.asarray(a2d_W, dtype=f32).T  # (1536, 1024)
    b_gate = (np.asarray(b_ih, dtype=f32) + np.asarray(b_hh, dtype=f32))
    b_gate_il = b_gate.reshape(4, H).T.reshape(128, 32).copy()  # interleaved
    c0_rm = c0v.reshape(128, 8).copy()

    # gate-interleaved, transposed LSTM weights (1024, 4096)
    wih_il_t = np.asarray(W_ih, dtype=f32).reshape(4, H, H).transpose(2, 1, 0).reshape(H, 4 * H)
    whh_il_t = np.asarray(W_hh, dtype=f32).reshape(4, H, H).transpose(2, 1, 0).reshape(H, 4 * H)

    out_Wf = np.asarray(out_W, dtype=f32)
    out_bf = np.asarray(out_b, dtype=f32)

    chunk_map = [[] for _ in range(N_CORES)]
    for j in range(NCH):
        chunk_map[j % N_CORES].append(j)
    for c in range(N_CORES):
        while len(chunk_map[c]) < SLOTS:
            chunk_map[c].append(-1)  # zero-pad slot

    in_maps = []
    for c in range(N_CORES):
        chunks = chunk_map[c]
        arr = np.zeros((SLOTS, 128, H), dtype=f32)
        b_row_c = np.full(VCORE, NEG, dtype=f32)
        for s, j in enumerate(chunks):
            if j < 0:
                continue
            lo, hi = j * 128, min((j + 1) * 128, V)
            n = hi - lo
            arr[s, :n] = out_Wf[lo:hi]
            b_row_c[s * 128 : s * 128 + n] = out_bf[lo:hi]
        # arr[s, i, r] -> wave tiles [w, p, kc, m] with r = p*8+kc, m = w*WM+m'
        w_out_c = np.ascontiguousarray(
            arr.reshape(VCORE, H).T          # (r, m)
            .reshape(128, 8, NW, WM)         # (p, kc, w, m')
            .transpose(2, 0, 1, 3)           # (w, p, kc, m')
        )
        a2d_wt_c = np.ascontiguousarray(
            a2d_wt_full[:, c * 128 : (c + 1) * 128]
            .reshape(KC_A, 128, 128)
            .transpose(1, 0, 2)
        )
        in_maps.append({
            "attn_in": attn_in,
            "attn_wt": attn_wt,
            "attn_b": attn_b_r,
            "enc": enc,
            "a2d_wt": a2d_wt_c,
            "a2d_b": np.asarray(a2d_b, dtype=f32)[c * 128 : (c + 1) * 128].reshape(128, 1).copy(),
            "wih_t": np.ascontiguousarray(wih_il_t[c * 128 : (c + 1) * 128]),
            "whh_t": np.ascontiguousarray(whh_il_t[c * 128 : (c + 1) * 128]),
            "h0_col": h0v[c * 128 : (c + 1) * 128].reshape(128, 1).copy(),
            "b_gates": b_gate_il,
            "c0_rm": c0_rm,
            "w_out": w_out_c,
            "b_row": b_row_c.reshape(1, VCORE),
        })
    return in_maps, chunk_map


def kernel(**inputs):
    outputs, _ = _run(inputs, trace=False)
    return outputs


def run_traced(inputs):
    """test-only entry: returns (outputs, BassKernelResults) with a HW trace."""
    return _run(inputs, trace=True)


def _run(inputs, trace):
    if "nc" not in _CACHE:
        _CACHE["nc"] = _build()
    nc = _CACHE["nc"]

    in_maps, chunk_map = _prep_inputs(**inputs)
    res = run_bass_kernel_spmd(
        nc, in_maps, core_ids=list(range(N_CORES)), trace=trace
    )
    results = res.results

    full = np.empty(NCH * 128, dtype=np.float32)
    for c in range(N_CORES):
        z_c = results[c]["z_out"].reshape(VCORE)
        for s, j in enumerate(chunk_map[c]):
            if j >= 0:
                full[j * 128 : (j + 1) * 128] = z_c[s * 128 : (s + 1) * 128]
    output = full[:V].reshape(1, V)

    h_new = results[0]["h_out"].reshape(1, 1, H).astype(np.float32)
    c_new = results[0]["c_out"].reshape(1, 1, H).astype(np.float32)
    attn_w = results[0]["aw_out"].reshape(1, L).astype(np.float32)
    return (output, h_new, c_new, attn_w), res


# revision 14
# speedup vs baseline: 1.6288x; 1.0728x over previous
"""Trainium2 Bass kernel for a single-step attention LSTM decoder (DecoderRNN).

Strategy (8 NeuronCores, SPMD):
  - The dominant cost is the vocab projection out_W (50257x1024 fp32, ~206 MB):
    sharded over vocab across the 8 cores (~26 MB/core), streamed in 8 waves of
    [contraction=1024, vocab=800] fp32r tiles used as the PE's moving operand
    (h stationary) -- ~45us of PE time, fully hidden under the DMA stream.
  - LSTM weights W_ih/W_hh (2x 4096x1024) are sharded over the contraction dim:
    each core computes partial pre-activation gates [1,4096] from its 128-wide
    slice of the hidden dim; a 16 KB AllReduce(add) produces full gates on all
    cores.
  - The a2d projection is likewise contraction-sharded for the LSTM input: each
    core only computes its own 128-slice of the a2d output (relu'd), which is
    exactly the slice its W_ih partial product needs.  No extra collective.
  - Attention (tiny) is computed replicated on every core.
  - log_softmax over the vocab: per-wave (max, sum-exp) stats on partition 0,
    combined locally, then a 64 B AllGather exchanges the 8 core-local stat
    pairs; each core computes the global logsumexp and normalizes its shard.
  - Embedding row gather + weight re-layout (transposes) happen on host inside
    kernel(); the device only streams what the math needs.
"""

import numpy as np

import concourse.bacc as bacc
import concourse.mybir as mybir
from concourse import tile
from concourse.bass_utils import run_bass_kernel_spmd

F32 = mybir.dt.float32
F32R = mybir.dt.float32r
AF = mybir.ActivationFunctionType
AX = mybir.AxisListType
N_CORES = 8
V, H, E, L = 50257, 1024, 512, 128
KC_A = (E + H) // 128          # 12 contraction chunks for attention / a2d
NCH = (V + 127) // 128         # 393 global vocab chunks
SLOTS = (NCH + N_CORES - 1) // N_CORES  # 50 vocab chunks per core
VCORE = SLOTS * 128            # 6400 vocab columns per core
NW, WM = 10, 640               # weight stream: 10 waves x 640 vocab cols
NEG = -1e30

_CACHE = {}


def _build():
    """Build + compile the SPMD Bass program (same on all cores)."""
    nc = bacc.Bacc(None, num_devices=N_CORES)

    din = {}
    for name, shape in [
        ("attn_b", [1, 128]),
        ("enc", [L, H]),                # natural layout
        ("a2d_wt", [128, KC_A, 128]),   # per-core slice of a2d_W.T
        ("a2d_b", [128, 1]),            # per-core slice
        ("b_gates", [128, 32]),         # gate-interleaved bias, natural [128,32]
        ("c0_rm", [128, 8]),            # c0 row-major-p layout
        ("b_row", [1, VCORE]),          # out_b shard (padded with -1e30)
    ]:
        din[name] = nc.dram_tensor(name, shape, F32, kind="ExternalInput")
    for name, shape in [
        ("attn_in", [128, KC_A]),       # [embed;h0] column-chunk layout
        ("attn_wt", [128, KC_A, 128]),  # attn_W.T re-laid [p, kc, l]
        ("wih_t", [128, 4 * H]),        # per-core k-slice, gate-interleaved cols
        ("whh_t", [128, 4 * H]),
        ("h0_col", [128, 1]),           # per-core k-slice of h0
        ("w_out", [NW, 128, 8, WM]),    # out weight stream tiles
    ]:
        din[name] = nc.dram_tensor(name, shape, F32R, kind="ExternalInput")

    z_out = nc.dram_tensor("z_out", [1, VCORE], F32, kind="ExternalOutput")
    h_out = nc.dram_tensor("h_out", [128, 8], F32, kind="ExternalOutput")
    c_out = nc.dram_tensor("c_out", [128, 8], F32, kind="ExternalOutput")
    aw_out = nc.dram_tensor("aw_out", [1, 128], F32, kind="ExternalOutput")

    rg = [list(range(N_CORES))]

    with tile.TileContext(nc) as tc:
        with (
            tc.tile_pool(name="const", bufs=1) as cp,
            tc.tile_pool(name="work", bufs=1) as wp,
            tc.tile_pool(name="wave", bufs=4) as wvp,
            tc.tile_pool(name="dram", bufs=1, space="DRAM") as dp,
        ):
            # ---- front weight loads (scalar HWDGE ring, before the big stream)
            attn_wt_sb = cp.tile([128, KC_A, 128], F32R, tag="attn_wt")
            nc.scalar.dma_start(attn_wt_sb[:], din["attn_wt"][:])
            a2d_wt_sb = cp.tile([128, KC_A, 128], F32, tag="a2d_wt")
            nc.scalar.dma_start(a2d_wt_sb[:], din["a2d_wt"][:])
            enc_sb = cp.tile([L, H], F32, tag="enc")
            nc.scalar.dma_start(enc_sb[:], din["enc"][:])
            wih_sb = cp.tile([128, 4 * H], F32R, tag="wih")
            nc.scalar.dma_start(wih_sb[:], din["wih_t"][:])
            whh_sb = cp.tile([128, 4 * H], F32R, tag="whh")
            nc.scalar.dma_start(whh_sb[:], din["whh_t"][:])

            # ---- small loads (sync ring)
            attn_in_sb = cp.tile([128, KC_A], F32R, tag="attn_in")
            nc.sync.dma_start(attn_in_sb[:], din["attn_in"][:])
            attn_b_sb = cp.tile([1, 128], F32, tag="attn_b")
            nc.sync.dma_start(attn_b_sb[:], din["attn_b"][:])
            a2d_b_sb = cp.tile([128, 1], F32, tag="a2d_b")
            nc.sync.dma_start(a2d_b_sb[:], din["a2d_b"][:])
            h0c_sb = cp.tile([128, 1], F32R, tag="h0c")
            nc.sync.dma_start(h0c_sb[:], din["h0_col"][:])
            bg_sb = cp.tile([128, 32], F32, tag="bg")
            nc.sync.dma_start(bg_sb[:], din["b_gates"][:])
            c0_sb = cp.tile([128, 8], F32, tag="c0")
            nc.sync.dma_start(c0_sb[:], din["c0_rm"][:])
            brow_sb = cp.tile([1, VCORE], F32, tag="brow")
            nc.sync.dma_start(brow_sb[:], din["b_row"][:])

            ones11 = cp.tile([1, 1], F32, tag="ones11")
            nc.vector.memset(ones11[:], 1.0)

            with tc.tile_pool(name="psmall", bufs=2, space="PSUM") as pps:
                # ================= attention (replicated) =================
                ps_a = pps.tile([1, 128], F32, tag="small")
                for kc in range(KC_A):
                    nc.tensor.matmul(
                        ps_a[:], attn_in_sb[:, kc : kc + 1], attn_wt_sb[:, kc, :],
                        start=(kc == 0), stop=(kc == KC_A - 1),
                    )
                za = wp.tile([1, 128], F32, tag="za")
                nc.vector.tensor_add(za[:], ps_a[:], attn_b_sb[:])
                # softmax over the 128 logits (all on partition 0)
                mneg = wp.tile([1, 1], F32, tag="mneg")
                nc.vector.reduce_max(mneg[:], za[:], axis=AX.X, negate=True)
                ew = wp.tile([1, 128], F32, tag="ew")
                ssum = wp.tile([1, 1], F32, tag="ssum")
                nc.scalar.activation(ew[:], za[:], AF.Exp,
                                     bias=mneg[:], accum_out=ssum[:])
                rsum = wp.tile([1, 1], F32, tag="rsum")
                nc.vector.reciprocal(rsum[:], ssum[:])
                w_row = wp.tile([1, 128], F32, tag="w_row")
                nc.vector.tensor_scalar_mul(w_row[:], ew[:], rsum[:])
                nc.sync.dma_start(aw_out[:], w_row[:])

                # transpose attn weights to a column: [1,128] -> [128,1]
                ps_w = pps.tile([128, 1], F32, tag="small")
                nc.tensor.matmul(ps_w[:], w_row[:], ones11[:], start=True, stop=True)
                wcol = wp.tile([128, 1], F32, tag="wcol")
                nc.vector.tensor_copy(wcol[:], ps_w[:])

                # attn_out = attn_weight @ enc_output, column-chunk layout [128,8]
                ps_c = pps.tile([128, 8], F32, tag="small")
                for j in range(8):
                    nc.tensor.matmul(
                        ps_c[:, j : j + 1], enc_sb[:, j * 128 : (j + 1) * 128],
                        wcol[:], start=True, stop=True,
                    )
                dec_in = wp.tile([128, KC_A], F32, tag="dec_in")
                nc.vector.tensor_copy(dec_in[:, 0:4], attn_in_sb[:, 0:4])
                nc.vector.tensor_copy(dec_in[:, 4:12], ps_c[:])

                # ===== a2d: only this core's 128-slice of the output =====
                ps_x = pps.tile([128, 1], F32, tag="small")
                for kc in range(KC_A):
                    nc.tensor.matmul(
                        ps_x[:], a2d_wt_sb[:, kc, :], dec_in[:, kc : kc + 1],
                        start=(kc == 0), stop=(kc == KC_A - 1),
                    )
                x_col = wp.tile([128, 1], F32R, tag="x_col")
                nc.scalar.activation(x_col[:], ps_x[:], AF.Relu, bias=a2d_b_sb[:])

            # ===== LSTM gates: partial [1, 4096], AllGather + local sum =====
            with tc.tile_pool(name="pgate", bufs=1, space="PSUM") as pg:
                g_dram = dp.tile([1, 4 * H], F32)
                g_part = wp.tile([1, 4 * H], F32, tag="g_part")
                ps_g = pg.tile([1, 4 * H], F32, tag="gates")
                for b in range(8):
                    lo = b * 512
                    nc.tensor.matmul(
                        ps_g[:, lo : lo + 512], x_col[:],
                        wih_sb[:, lo : lo + 512], start=True, stop=False,
                    )
                    nc.tensor.matmul(
                        ps_g[:, lo : lo + 512], h0c_sb[:],
                        whh_sb[:, lo : lo + 512], start=False, stop=True,
                    )
                # PSUM can't be DMA'd; copy out on two engines in parallel
                nc.vector.tensor_copy(g_part[:, 0:2048], ps_g[:, 0:2048])
                nc.scalar.activation(g_part[:, 2048:4096], ps_g[:, 2048:4096],
                                     AF.Copy)
                nc.sync.dma_start(g_dram[:], g_part[:])
                g_ag = dp.tile([N_CORES, 4 * H], F32)
                nc.gpsimd.collective_compute(
                    "AllGather", mybir.AluOpType.bypass, replica_groups=rg,
                    ins=[g_dram.opt()], outs=[g_ag.opt()],
                )

                # gather partials as [p, rank, 32] and sum over ranks
                g8_sb = wp.tile([128, N_CORES, 32], F32, tag="g8_sb")
                nc.sync.dma_start(
                    g8_sb[:], g_ag[:].rearrange("r (p j) -> p r j", p=128))
                g_sum = wp.tile([128, 32], F32, tag="g_sum")
                nc.vector.reduce_sum(
                    g_sum[:], g8_sb[:].rearrange("p r j -> p j r"), axis=AX.X)
                t1 = wp.tile([128, 32], F32, tag="t1")
                nc.vector.tensor_add(t1[:], g_sum[:], bg_sb[:])
                t1v = t1[:].rearrange("p (a b) -> p a b", b=4)

                sig_i = wp.tile([128, 8], F32, tag="sig_i")
                nc.scalar.activation(sig_i[:], t1v[:, :, 0], AF.Sigmoid)
                sig_f = wp.tile([128, 8], F32, tag="sig_f")
                nc.scalar.activation(sig_f[:], t1v[:, :, 1], AF.Sigmoid)
                tanh_g = wp.tile([128, 8], F32, tag="tanh_g")
                nc.scalar.activation(tanh_g[:], t1v[:, :, 2], AF.Tanh)
                sig_o = wp.tile([128, 8], F32, tag="sig_o")
                nc.scalar.activation(sig_o[:], t1v[:, :, 3], AF.Sigmoid)

                c_new = wp.tile([128, 8], F32, tag="c_new")
                tmp = wp.tile([128, 8], F32, tag="tmp")
                nc.vector.tensor_mul(tmp[:], sig_f[:], c0_sb[:])
                nc.vector.tensor_mul(c_new[:], sig_i[:], tanh_g[:])
                nc.vector.tensor_add(c_new[:], c_new[:], tmp[:])
                tanh_c = wp.tile([128, 8], F32, tag="tanh_c")
                nc.scalar.activation(tanh_c[:], c_new[:], AF.Tanh)
                h_new = wp.tile([128, 8], F32, tag="h_new")
                nc.vector.tensor_mul(h_new[:], sig_o[:], tanh_c[:])
                nc.sync.dma_start(h_out[:], h_new[:])
                nc.sync.dma_start(c_out[:], c_new[:])
                h_r = wp.tile([128, 8], F32R, tag="h_r")
                nc.vector.tensor_copy(h_r[:], h_new[:])

            # ========== vocab projection: 8 waves of [1024 x 800] ==========
            z_row = wp.tile([1, VCORE], F32, tag="z_row")
            negm_all = wp.tile([1, NW], F32, tag="negm_all")
            s_all = wp.tile([1, NW], F32, tag="s_all")
            with tc.tile_pool(name="pz", bufs=2, space="PSUM") as zp:
                for w in range(NW):
                    wv = wvp.tile([128, 8, WM], F32R, tag="wv")
                    nc.scalar.dma_start(wv[:], din["w_out"][w])
                    ps = zp.tile([1, WM], F32, tag="zps")
                    for lo, n in ((0, 512), (512, WM - 512)):
                        for kc in range(8):
                            nc.tensor.matmul(
                                ps[:, lo : lo + n], h_r[:, kc : kc + 1],
                                wv[:, kc, lo : lo + n],
                                start=(kc == 0), stop=(kc == 7),
                            )
                    seg = z_row[:, w * WM : (w + 1) * WM]
                    nc.vector.tensor_add(
                        seg, ps[:], brow_sb[:, w * WM : (w + 1) * WM])
                    nc.vector.reduce_max(
                        negm_all[:, w : w + 1], seg, axis=AX.X, negate=True)
                    e_scr = wp.tile([1, WM], F32, tag="e_scr")
                    nc.scalar.activation(
                        e_scr[:], seg, AF.Exp,
                        bias=negm_all[:, w : w + 1],
                        accum_out=s_all[:, w : w + 1],
                    )

                # local stats:  m_loc = max_w m_w,  s_loc = sum_w s_w*e^(m_w-m_loc)
                negm_loc = wp.tile([1, 1], F32, tag="negm_loc")
                nc.vector.tensor_reduce(
                    negm_loc[:], negm_all[:], axis=AX.X, op=mybir.AluOpType.min)
                terms = wp.tile([1, NW], F32, tag="terms")
                nc.scalar.activation(terms[:], negm_all[:], AF.Exp,
                                     bias=negm_loc[:], scale=-1.0)
                nc.vector.tensor_mul(terms[:], terms[:], s_all[:])
                s_loc = wp.tile([1, 1], F32, tag="s_loc")
                nc.vector.reduce_sum(s_loc[:], terms[:], axis=AX.X)
                stats = wp.tile([1, 2], F32, tag="stats")
                nc.vector.tensor_scalar_mul(stats[:, 0:1], negm_loc[:], -1.0)
                nc.vector.tensor_copy(stats[:, 1:2], s_loc[:])

                st_in = dp.tile([1, 2], F32)
                st_all = dp.tile([N_CORES, 2], F32)
                nc.sync.dma_start(st_in[:], stats[:])
                nc.gpsimd.collective_compute(
                    "AllGather", mybir.AluOpType.bypass, replica_groups=rg,
                    ins=[st_in.opt()], outs=[st_all.opt()],
                )
                st_sb = wp.tile([1, N_CORES, 2], F32, tag="st_sb")
                nc.sync.dma_start(st_sb[:], st_all[:])

                mg = wp.tile([1, 1], F32, tag="mg")
                nc.vector.reduce_max(mg[:], st_sb[:, :, 0], axis=AX.X)
                negmg = wp.tile([1, 1], F32, tag="negmg")
                nc.vector.tensor_scalar_mul(negmg[:], mg[:], -1.0)
                terms2 = wp.tile([1, N_CORES], F32, tag="terms2")
                nc.scalar.activation(terms2[:], st_sb[:, :, 0], AF.Exp,
                                     bias=negmg[:])
                nc.vector.tensor_mul(terms2[:], terms2[:], st_sb[:, :, 1])
                stot = wp.tile([1, 1], F32, tag="stot")
                nc.vector.reduce_sum(stot[:], terms2[:], axis=AX.X)
                lnst = wp.tile([1, 1], F32, tag="lnst")
                nc.scalar.activation(lnst[:], stot[:], AF.Ln)
                neglse = wp.tile([1, 1], F32, tag="neglse")
                nc.vector.tensor_sub(neglse[:], negmg[:], lnst[:])

                # z -= lse, split across DVE and ACT, then store
                half = VCORE // 2
                nc.vector.tensor_scalar_add(
                    z_row[:, 0:half], z_row[:, 0:half], neglse[:])
                nc.scalar.activation(
                    z_row[:, half:VCORE], z_row[:, half:VCORE], AF.Identity,
                    bias=neglse[:])
                nc.sync.dma_start(z_out[:], z_row[:])

    nc.compile()
    return nc


def _prep_inputs(input_tok, h0, c0, enc_output, emb_table, attn_W, attn_b,
                 a2d_W, a2d_b, W_ih, W_hh, b_ih, b_hh, out_W, out_b):
    """Host-side sharding / re-layout. Returns per-core input maps + chunk map."""
    f32 = np.float32
    tok = int(np.asarray(input_tok).reshape(-1)[0])
    embed = np.asarray(emb_table[tok], dtype=f32).reshape(E)
    h0v = np.asarray(h0, dtype=f32).reshape(H)
    c0v = np.asarray(c0, dtype=f32).reshape(H)
    enc = np.ascontiguousarray(np.asarray(enc_output, dtype=f32))

    attn_in = np.concatenate([embed, h0v]).reshape(KC_A, 128).T.copy()  # [128,12]
    attn_wt = (
        np.asarray(attn_W, dtype=f32).T.reshape(KC_A, 128, 128).transpose(1, 0, 2).copy()
    )  # [128, kc, l]
    attn_b_r = np.asarray(attn_b, dtype=f32).reshape(1, 128).copy()

    a2d_wt_full = np.asarray(a2d_W, dtype=f32).T  # (1536, 1024)
    b_gate = (np.asarray(b_ih, dtype=f32) + np.asarray(b_hh, dtype=f32))
    b_gate_il = b_gate.reshape(4, H).T.reshape(128, 32).copy()  # interleaved
    c0_rm = c0v.reshape(128, 8).copy()

    # gate-interleaved, transposed LSTM weights (1024, 4096)
    wih_il_t = np.asarray(W_ih, dtype=f32).reshape(4, H, H).transpose(2, 1, 0).reshape(H, 4 * H)
    whh_il_t = np.asarray(W_hh, dtype=f32).reshape(4, H, H).transpose(2, 1, 0).reshape(H, 4 * H)

    out_Wf = np.asarray(out_W, dtype=f32)
    out_bf = np.asarray(out_b, dtype=f32)

    chunk_map = [[] for _ in range(N_CORES)]
    for j in range(NCH):
        chunk_map[j % N_CORES].append(j)
    for c in range(N_CORES):
        while len(chunk_map[c]) < SLOTS:
            chunk_map[c].append(-1)  # zero-pad slot

    in_maps = []
    for c in range(N_CORES):
        chunks = chunk_map[c]
        arr = np.zeros((SLOTS, 128, H), dtype=f32)
        b_row_c = np.full(VCORE, NEG, dtype=f32)
        for s, j in enumerate(chunks):
            if j < 0:
                continue
            lo, hi = j * 128, min((j + 1) * 128, V)
            n = hi - lo
            arr[s, :n] = out_Wf[lo:hi]
            b_row_c[s * 128 : s * 128 + n] = out_bf[lo:hi]
        # arr[s, i, r] -> wave tiles [w, p, kc, m] with r = p*8+kc, m = w*WM+m'
        w_out_c = np.ascontiguousarray(
            arr.reshape(VCORE, H).T          # (r, m)
            .reshape(128, 8, NW, WM)         # (p, kc, w, m')
            .transpose(2, 0, 1, 3)           # (w, p, kc, m')
        )
        a2d_wt_c = np.ascontiguousarray(
            a2d_wt_full[:, c * 128 : (c + 1) * 128]
            .reshape(KC_A, 128, 128)
            .transpose(1, 0, 2)
        )
        in_maps.append({
            "attn_in": attn_in,
            "attn_wt": attn_wt,
            "attn_b": attn_b_r,
            "enc": enc,
            "a2d_wt": a2d_wt_c,
            "a2d_b": np.asarray(a2d_b, dtype=f32)[c * 128 : (c + 1) * 128].reshape(128, 1).copy(),
            "wih_t": np.ascontiguousarray(wih_il_t[c * 128 : (c + 1) * 128]),
            "whh_t": np.ascontiguousarray(whh_il_t[c * 128 : (c + 1) * 128]),
            "h0_col": h0v[c * 128 : (c + 1) * 128].reshape(128, 1).copy(),
            "b_gates": b_gate_il,
            "c0_rm": c0_rm,
            "w_out": w_out_c,
            "b_row": b_row_c.reshape(1, VCORE),
        })
    return in_maps, chunk_map


def kernel(**inputs):
    outputs, _ = _run(inputs, trace=False)
    return outputs


def run_traced(inputs):
    """test-only entry: returns (outputs, BassKernelResults) with a HW trace."""
    return _run(inputs, trace=True)


def _run(inputs, trace):
    if "nc" not in _CACHE:
        _CACHE["nc"] = _build()
    nc = _CACHE["nc"]

    in_maps, chunk_map = _prep_inputs(**inputs)
    res = run_bass_kernel_spmd(
        nc, in_maps, core_ids=list(range(N_CORES)), trace=trace
    )
    results = res.results

    full = np.empty(NCH * 128, dtype=np.float32)
    for c in range(N_CORES):
        z_c = results[c]["z_out"].reshape(VCORE)
        for s, j in enumerate(chunk_map[c]):
            if j >= 0:
                full[j * 128 : (j + 1) * 128] = z_c[s * 128 : (s + 1) * 128]
    output = full[:V].reshape(1, V)

    h_new = results[0]["h_out"].reshape(1, 1, H).astype(np.float32)
    c_new = results[0]["c_out"].reshape(1, 1, H).astype(np.float32)
    attn_w = results[0]["aw_out"].reshape(1, L).astype(np.float32)
    return (output, h_new, c_new, attn_w), res


# revision 15
# speedup vs baseline: 1.6778x; 1.0301x over previous
"""Trainium2 Bass kernel for a single-step attention LSTM decoder (DecoderRNN).

Strategy (8 NeuronCores, SPMD):
  - The dominant cost is the vocab projection out_W (50257x1024 fp32, ~206 MB):
    sharded over vocab across the 8 cores (~26 MB/core), streamed in 8 waves of
    [contraction=1024, vocab=800] fp32r tiles used as the PE's moving operand
    (h stationary) -- ~45us of PE time, fully hidden under the DMA stream.
  - LSTM weights W_ih/W_hh (2x 4096x1024) are sharded over the contraction dim:
    each core computes partial pre-activation gates [1,4096] from its 128-wide
    slice of the hidden dim; a 16 KB AllReduce(add) produces full gates on all
    cores.
  - The a2d projection is likewise contraction-sharded for the LSTM input: each
    core only computes its own 128-slice of the a2d output (relu'd), which is
    exactly the slice its W_ih partial product needs.  No extra collective.
  - Attention (tiny) is computed replicated on every core.
  - log_softmax over the vocab: per-wave (max, sum-exp) stats on partition 0,
    combined locally, then a 64 B AllGather exchanges the 8 core-local stat
    pairs; each core computes the global logsumexp and normalizes its shard.
  - Embedding row gather + weight re-layout (transposes) happen on host inside
    kernel(); the device only streams what the math needs.
"""

import numpy as np

import concourse.bacc as bacc
import concourse.mybir as mybir
from concourse import tile
from concourse.bass_utils import run_bass_kernel_spmd

F32 = mybir.dt.float32
F32R = mybir.dt.float32r
AF = mybir.ActivationFunctionType
AX = mybir.AxisListType
N_CORES = 8
V, H, E, L = 50257, 1024, 512, 128
KC_A = (E + H) // 128          # 12 contraction chunks for attention / a2d
NCH = (V + 127) // 128         # 393 global vocab chunks
SLOTS = (NCH + N_CORES - 1) // N_CORES  # 50 vocab chunks per core
VCORE = SLOTS * 128            # 6400 vocab columns per core
NW, WM = 10, 640               # weight stream: 10 waves x 640 vocab cols
NEG = -1e30

_CACHE = {}


def _build():
    """Build + compile the SPMD Bass program (same on all cores)."""
    nc = bacc.Bacc(None, num_devices=N_CORES)

    din = {}
    for name, shape in [
        ("attn_b", [1, 128]),
        ("a2d_b", [128, 1]),            # per-core slice
        ("b_gates", [128, 32]),         # gate-interleaved bias, natural [128,32]
        ("c0_rm", [128, 8]),            # c0 row-major-p layout
        ("b_row", [1, VCORE]),          # out_b shard (padded with -1e30)
    ]:
        din[name] = nc.dram_tensor(name, shape, F32, kind="ExternalInput")
    for name, shape in [
        ("attn_in", [128, KC_A]),       # [embed;h0] column-chunk layout
        ("attn_wt", [128, KC_A, 128]),  # attn_W.T re-laid [p, kc, l]
        ("enc", [L, H]),                # natural layout
        ("a2d_wt", [128, KC_A, 128]),   # per-core slice of a2d_W.T
        ("wih_t", [128, 4 * H]),        # per-core k-slice, gate-interleaved cols
        ("whh_t", [128, 4 * H]),
        ("h0_col", [128, 1]),           # per-core k-slice of h0
        ("w_out", [NW, 128, 8, WM]),    # out weight stream tiles
    ]:
        din[name] = nc.dram_tensor(name, shape, F32R, kind="ExternalInput")

    z_out = nc.dram_tensor("z_out", [1, VCORE], F32, kind="ExternalOutput")
    h_out = nc.dram_tensor("h_out", [128, 8], F32, kind="ExternalOutput")
    c_out = nc.dram_tensor("c_out", [128, 8], F32, kind="ExternalOutput")
    aw_out = nc.dram_tensor("aw_out", [1, 128], F32, kind="ExternalOutput")

    rg = [list(range(N_CORES))]

    with tile.TileContext(nc) as tc:
        with (
            tc.tile_pool(name="const", bufs=1) as cp,
            tc.tile_pool(name="work", bufs=1) as wp,
            tc.tile_pool(name="wave", bufs=5) as wvp,
            tc.tile_pool(name="bias", bufs=2) as bp,
            tc.tile_pool(name="dram", bufs=1, space="DRAM") as dp,
        ):
            # ---- front weight loads (scalar HWDGE ring, before the big stream)
            attn_wt_sb = cp.tile([128, KC_A, 128], F32R, tag="attn_wt")
            nc.scalar.dma_start(attn_wt_sb[:], din["attn_wt"][:])
            a2d_wt_sb = cp.tile([128, KC_A, 128], F32R, tag="a2d_wt")
            nc.scalar.dma_start(a2d_wt_sb[:], din["a2d_wt"][:])
            enc_sb = cp.tile([L, H], F32R, tag="enc")
            nc.scalar.dma_start(enc_sb[:], din["enc"][:])
            wih_sb = cp.tile([128, 4 * H], F32R, tag="wih")
            nc.scalar.dma_start(wih_sb[:], din["wih_t"][:])
            whh_sb = cp.tile([128, 4 * H], F32R, tag="whh")
            nc.scalar.dma_start(whh_sb[:], din["whh_t"][:])

            # ---- small loads (sync ring)
            attn_in_sb = cp.tile([128, KC_A], F32R, tag="attn_in")
            nc.sync.dma_start(attn_in_sb[:], din["attn_in"][:])
            attn_b_sb = cp.tile([1, 128], F32, tag="attn_b")
            nc.sync.dma_start(attn_b_sb[:], din["attn_b"][:])
            a2d_b_sb = cp.tile([128, 1], F32, tag="a2d_b")
            nc.sync.dma_start(a2d_b_sb[:], din["a2d_b"][:])
            h0c_sb = cp.tile([128, 1], F32R, tag="h0c")
            nc.sync.dma_start(h0c_sb[:], din["h0_col"][:])
            bg_sb = cp.tile([128, 32], F32, tag="bg")
            nc.sync.dma_start(bg_sb[:], din["b_gates"][:])
            c0_sb = cp.tile([128, 8], F32, tag="c0")
            nc.sync.dma_start(c0_sb[:], din["c0_rm"][:])

            ones11 = cp.tile([1, 1], F32, tag="ones11")
            nc.vector.memset(ones11[:], 1.0)

            with tc.tile_pool(name="psmall", bufs=2, space="PSUM") as pps:
                # ================= attention (replicated) =================
                ps_a = pps.tile([1, 128], F32, tag="small")
                for kc in range(KC_A):
                    nc.tensor.matmul(
                        ps_a[:], attn_in_sb[:, kc : kc + 1], attn_wt_sb[:, kc, :],
                        start=(kc == 0), stop=(kc == KC_A - 1),
                    )
                za = wp.tile([1, 128], F32, tag="za")
                nc.vector.tensor_add(za[:], ps_a[:], attn_b_sb[:])
                # softmax over the 128 logits (all on partition 0)
                mneg = wp.tile([1, 1], F32, tag="mneg")
                nc.vector.reduce_max(mneg[:], za[:], axis=AX.X, negate=True)
                ew = wp.tile([1, 128], F32, tag="ew")
                ssum = wp.tile([1, 1], F32, tag="ssum")
                nc.scalar.activation(ew[:], za[:], AF.Exp,
                                     bias=mneg[:], accum_out=ssum[:])
                rsum = wp.tile([1, 1], F32, tag="rsum")
                nc.vector.reciprocal(rsum[:], ssum[:])
                w_row = wp.tile([1, 128], F32, tag="w_row")
                nc.vector.tensor_scalar_mul(w_row[:], ew[:], rsum[:])
                nc.sync.dma_start(aw_out[:], w_row[:])

                # transpose attn weights to a column, duplicated to 2 cols so
                # downstream fp32r matmuls have an even moving free dim
                ps_w = pps.tile([128, 1], F32, tag="small")
                nc.tensor.matmul(ps_w[:], w_row[:], ones11[:], start=True, stop=True)
                wcol2 = wp.tile([128, 2], F32R, tag="wcol2")
                nc.vector.tensor_copy(wcol2[:, 0:1], ps_w[:])
                nc.vector.tensor_copy(wcol2[:, 1:2], ps_w[:])

                # attn_out = attn_weight @ enc_output (fp32r, J=2 duplicated)
                ps_c = pps.tile([128, 16], F32, tag="small")
                for j in range(8):
                    nc.tensor.matmul(
                        ps_c[:, 2 * j : 2 * j + 2],
                        enc_sb[:, j * 128 : (j + 1) * 128],
                        wcol2[:], start=True, stop=True,
                    )
                # dec duplicated to column pairs for the J=2 a2d matmuls
                dec2 = wp.tile([128, 2 * KC_A], F32R, tag="dec2")
                nc.vector.tensor_copy(dec2[:, 0:8:2], attn_in_sb[:, 0:4])
                nc.vector.tensor_copy(dec2[:, 1:8:2], attn_in_sb[:, 0:4])
                nc.vector.tensor_copy(dec2[:, 8::2], ps_c[:, 0::2])
                nc.vector.tensor_copy(dec2[:, 9::2], ps_c[:, 0::2])

                # ===== a2d: only this core's 128-slice of the output =====
                ps_x = pps.tile([128, 2], F32, tag="small")
                for kc in range(KC_A):
                    nc.tensor.matmul(
                        ps_x[:], a2d_wt_sb[:, kc, :], dec2[:, 2 * kc : 2 * kc + 2],
                        start=(kc == 0), stop=(kc == KC_A - 1),
                    )
                x_col = wp.tile([128, 1], F32R, tag="x_col")
                nc.scalar.activation(x_col[:], ps_x[:, 0:1], AF.Relu, bias=a2d_b_sb[:])

            # ===== LSTM gates: partial [1, 4096], AllGather + local sum =====
            with tc.tile_pool(name="pgate", bufs=1, space="PSUM") as pg:
                g_dram = dp.tile([1, 4 * H], F32)
                g_part = wp.tile([1, 4 * H], F32, tag="g_part")
                ps_g = pg.tile([1, 4 * H], F32, tag="gates")
                for b in range(8):
                    lo = b * 512
                    nc.tensor.matmul(
                        ps_g[:, lo : lo + 512], x_col[:],
                        wih_sb[:, lo : lo + 512], start=True, stop=False,
                    )
                    nc.tensor.matmul(
                        ps_g[:, lo : lo + 512], h0c_sb[:],
                        whh_sb[:, lo : lo + 512], start=False, stop=True,
                    )
                # PSUM can't be DMA'd; copy out on two engines in parallel
                nc.vector.tensor_copy(g_part[:, 0:2048], ps_g[:, 0:2048])
                nc.scalar.activation(g_part[:, 2048:4096], ps_g[:, 2048:4096],
                                     AF.Copy)
                nc.sync.dma_start(g_dram[:], g_part[:])
                g_ag = dp.tile([N_CORES, 4 * H], F32)
                nc.gpsimd.collective_compute(
                    "AllGather", mybir.AluOpType.bypass, replica_groups=rg,
                    ins=[g_dram.opt()], outs=[g_ag.opt()],
                )

                # gather partials as [p, rank, 32] and sum over ranks
                g8_sb = wp.tile([128, N_CORES, 32], F32, tag="g8_sb")
                nc.sync.dma_start(
                    g8_sb[:], g_ag[:].rearrange("r (p j) -> p r j", p=128))
                g_sum = wp.tile([128, 32], F32, tag="g_sum")
                nc.vector.reduce_sum(
                    g_sum[:], g8_sb[:].rearrange("p r j -> p j r"), axis=AX.X)
                t1 = wp.tile([128, 32], F32, tag="t1")
                nc.vector.tensor_add(t1[:], g_sum[:], bg_sb[:])
                t1v = t1[:].rearrange("p (a b) -> p a b", b=4)

                sig_i = wp.tile([128, 8], F32, tag="sig_i")
                nc.scalar.activation(sig_i[:], t1v[:, :, 0], AF.Sigmoid)
                sig_f = wp.tile([128, 8], F32, tag="sig_f")
                nc.scalar.activation(sig_f[:], t1v[:, :, 1], AF.Sigmoid)
                tanh_g = wp.tile([128, 8], F32, tag="tanh_g")
                nc.scalar.activation(tanh_g[:], t1v[:, :, 2], AF.Tanh)
                sig_o = wp.tile([128, 8], F32, tag="sig_o")
                nc.scalar.activation(sig_o[:], t1v[:, :, 3], AF.Sigmoid)

                c_new = wp.tile([128, 8], F32, tag="c_new")
                tmp = wp.tile([128, 8], F32, tag="tmp")
                nc.vector.tensor_mul(tmp[:], sig_f[:], c0_sb[:])
                nc.vector.tensor_mul(c_new[:], sig_i[:], tanh_g[:])
                nc.vector.tensor_add(c_new[:], c_new[:], tmp[:])
                tanh_c = wp.tile([128, 8], F32, tag="tanh_c")
                nc.scalar.activation(tanh_c[:], c_new[:], AF.Tanh)
                h_new = wp.tile([128, 8], F32, tag="h_new")
                nc.vector.tensor_mul(h_new[:], sig_o[:], tanh_c[:])
                nc.sync.dma_start(h_out[:], h_new[:])
                nc.sync.dma_start(c_out[:], c_new[:])
                h_r = wp.tile([128, 8], F32R, tag="h_r")
                nc.vector.tensor_copy(h_r[:], h_new[:])

            # ========== vocab projection: 8 waves of [1024 x 800] ==========
            z_row = wp.tile([1, VCORE], F32, tag="z_row")
            negm_all = wp.tile([1, NW], F32, tag="negm_all")
            s_all = wp.tile([1, NW], F32, tag="s_all")
            with tc.tile_pool(name="pz", bufs=2, space="PSUM") as zp:
                for w in range(NW):
                    wv = wvp.tile([128, 8, WM], F32R, tag="wv")
                    nc.scalar.dma_start(wv[:], din["w_out"][w])
                    bw = bp.tile([1, WM], F32, tag="bw")
                    nc.sync.dma_start(bw[:], din["b_row"][:, w * WM : (w + 1) * WM])
                    ps = zp.tile([1, WM], F32, tag="zps")
                    for lo, n in ((0, 512), (512, WM - 512)):
                        for kc in range(8):
                            nc.tensor.matmul(
                                ps[:, lo : lo + n], h_r[:, kc : kc + 1],
                                wv[:, kc, lo : lo + n],
                                start=(kc == 0), stop=(kc == 7),
                            )
                    seg = z_row[:, w * WM : (w + 1) * WM]
                    nc.vector.tensor_add(seg, ps[:], bw[:])
                    nc.vector.reduce_max(
                        negm_all[:, w : w + 1], seg, axis=AX.X, negate=True)
                    e_scr = wp.tile([1, WM], F32, tag="e_scr")
                    nc.scalar.activation(
                        e_scr[:], seg, AF.Exp,
                        bias=negm_all[:, w : w + 1],
                        accum_out=s_all[:, w : w + 1],
                    )

                # local stats:  m_loc = max_w m_w,  s_loc = sum_w s_w*e^(m_w-m_loc)
                negm_loc = wp.tile([1, 1], F32, tag="negm_loc")
                nc.vector.tensor_reduce(
                    negm_loc[:], negm_all[:], axis=AX.X, op=mybir.AluOpType.min)
                terms = wp.tile([1, NW], F32, tag="terms")
                nc.scalar.activation(terms[:], negm_all[:], AF.Exp,
                                     bias=negm_loc[:], scale=-1.0)
                nc.vector.tensor_mul(terms[:], terms[:], s_all[:])
                s_loc = wp.tile([1, 1], F32, tag="s_loc")
                nc.vector.reduce_sum(s_loc[:], terms[:], axis=AX.X)
                stats = wp.tile([1, 2], F32, tag="stats")
                nc.vector.tensor_scalar_mul(stats[:, 0:1], negm_loc[:], -1.0)
                nc.vector.tensor_copy(stats[:, 1:2], s_loc[:])

                st_in = dp.tile([1, 2], F32)
                st_all = dp.tile([N_CORES, 2], F32)
                nc.sync.dma_start(st_in[:], stats[:])
                nc.gpsimd.collective_compute(
                    "AllGather", mybir.AluOpType.bypass, replica_groups=rg,
                    ins=[st_in.opt()], outs=[st_all.opt()],
                )
                st_sb = wp.tile([1, N_CORES, 2], F32, tag="st_sb")
                nc.sync.dma_start(st_sb[:], st_all[:])

                mg = wp.tile([1, 1], F32, tag="mg")
                nc.vector.reduce_max(mg[:], st_sb[:, :, 0], axis=AX.X)
                negmg = wp.tile([1, 1], F32, tag="negmg")
                nc.vector.tensor_scalar_mul(negmg[:], mg[:], -1.0)
                terms2 = wp.tile([1, N_CORES], F32, tag="terms2")
                nc.scalar.activation(terms2[:], st_sb[:, :, 0], AF.Exp,
                                     bias=negmg[:])
                nc.vector.tensor_mul(terms2[:], terms2[:], st_sb[:, :, 1])
                stot = wp.tile([1, 1], F32, tag="stot")
                nc.vector.reduce_sum(stot[:], terms2[:], axis=AX.X)
                lnst = wp.tile([1, 1], F32, tag="lnst")
                nc.scalar.activation(lnst[:], stot[:], AF.Ln)
                neglse = wp.tile([1, 1], F32, tag="neglse")
                nc.vector.tensor_sub(neglse[:], negmg[:], lnst[:])

                # z -= lse, split across DVE and ACT, then store
                half = VCORE // 2
                nc.vector.tensor_scalar_add(
                    z_row[:, 0:half], z_row[:, 0:half], neglse[:])
                nc.scalar.activation(
                    z_row[:, half:VCORE], z_row[:, half:VCORE], AF.Identity,
                    bias=neglse[:])
                nc.sync.dma_start(z_out[:], z_row[:])

    nc.compile()
    return nc


def _prep_inputs(input_tok, h0, c0, enc_output, emb_table, attn_W, attn_b,
                 a2d_W, a2d_b, W_ih, W_hh, b_ih, b_hh, out_W, out_b):
    """Host-side sharding / re-layout. Returns per-core input maps + chunk map."""
    f32 = np.float32
    tok = int(np.asarray(input_tok).reshape(-1)[0])
    embed = np.asarray(emb_table[tok], dtype=f32).reshape(E)
    h0v = np.asarray(h0, dtype=f32).reshape(H)
    c0v = np.asarray(c0, dtype=f32).reshape(H)
    enc = np.ascontiguousarray(np.asarray(enc_output, dtype=f32))

    attn_in = np.concatenate([embed, h0v]).reshape(KC_A, 128).T.copy()  # [128,12]
    attn_wt = (
        np.asarray(attn_W, dtype=f32).T.reshape(KC_A, 128, 128).transpose(1, 0, 2).copy()
    )  # [128, kc, l]
    attn_b_r = np.asarray(attn_b, dtype=f32).reshape(1, 128).copy()

    a2d_wt_full = np.asarray(a2d_W, dtype=f32).T  # (1536, 1024)
    b_gate = (np.asarray(b_ih, dtype=f32) + np.asarray(b_hh, dtype=f32))
    b_gate_il = b_gate.reshape(4, H).T.reshape(128, 32).copy()  # interleaved
    c0_rm = c0v.reshape(128, 8).copy()

    # gate-interleaved, transposed LSTM weights (1024, 4096)
    wih_il_t = np.asarray(W_ih, dtype=f32).reshape(4, H, H).transpose(2, 1, 0).reshape(H, 4 * H)
    whh_il_t = np.asarray(W_hh, dtype=f32).reshape(4, H, H).transpose(2, 1, 0).reshape(H, 4 * H)

    out_Wf = np.asarray(out_W, dtype=f32)
    out_bf = np.asarray(out_b, dtype=f32)

    chunk_map = [[] for _ in range(N_CORES)]
    for j in range(NCH):
        chunk_map[j % N_CORES].append(j)
    for c in range(N_CORES):
        while len(chunk_map[c]) < SLOTS:
            chunk_map[c].append(-1)  # zero-pad slot

    in_maps = []
    for c in range(N_CORES):
        chunks = chunk_map[c]
        arr = np.zeros((SLOTS, 128, H), dtype=f32)
        b_row_c = np.full(VCORE, NEG, dtype=f32)
        for s, j in enumerate(chunks):
            if j < 0:
                continue
            lo, hi = j * 128, min((j + 1) * 128, V)
            n = hi - lo
            arr[s, :n] = out_Wf[lo:hi]
            b_row_c[s * 128 : s * 128 + n] = out_bf[lo:hi]
        # arr[s, i, r] -> wave tiles [w, p, kc, m] with r = p*8+kc, m = w*WM+m'
        w_out_c = np.ascontiguousarray(
            arr.reshape(VCORE, H).T          # (r, m)
            .reshape(128, 8, NW, WM)         # (p, kc, w, m')
            .transpose(2, 0, 1, 3)           # (w, p, kc, m')
        )
        a2d_wt_c = np.ascontiguousarray(
            a2d_wt_full[:, c * 128 : (c + 1) * 128]
            .reshape(KC_A, 128, 128)
            .transpose(1, 0, 2)
        )
        in_maps.append({
            "attn_in": attn_in,
            "attn_wt": attn_wt,
            "attn_b": attn_b_r,
            "enc": enc,
            "a2d_wt": a2d_wt_c,
            "a2d_b": np.asarray(a2d_b, dtype=f32)[c * 128 : (c + 1) * 128].reshape(128, 1).copy(),
            "wih_t": np.ascontiguousarray(wih_il_t[c * 128 : (c + 1) * 128]),
            "whh_t": np.ascontiguousarray(whh_il_t[c * 128 : (c + 1) * 128]),
            "h0_col": h0v[c * 128 : (c + 1) * 128].reshape(128, 1).copy(),
            "b_gates": b_gate_il,
            "c0_rm": c0_rm,
            "w_out": w_out_c,
            "b_row": b_row_c.reshape(1, VCORE),
        })
    return in_maps, chunk_map


def kernel(**inputs):
    outputs, _ = _run(inputs, trace=False)
    return outputs


def run_traced(inputs):
    """test-only entry: returns (outputs, BassKernelResults) with a HW trace."""
    return _run(inputs, trace=True)


def _run(inputs, trace):
    if "nc" not in _CACHE:
        _CACHE["nc"] = _build()
    nc = _CACHE["nc"]

    in_maps, chunk_map = _prep_inputs(**inputs)
    res = run_bass_kernel_spmd(
        nc, in_maps, core_ids=list(range(N_CORES)), trace=trace
    )
    results = res.results

    full = np.empty(NCH * 128, dtype=np.float32)
    for c in range(N_CORES):
        z_c = results[c]["z_out"].reshape(VCORE)
        for s, j in enumerate(chunk_map[c]):
            if j >= 0:
                full[j * 128 : (j + 1) * 128] = z_c[s * 128 : (s + 1) * 128]
    output = full[:V].reshape(1, V)

    h_new = results[0]["h_out"].reshape(1, 1, H).astype(np.float32)
    c_new = results[0]["c_out"].reshape(1, 1, H).astype(np.float32)
    attn_w = results[0]["aw_out"].reshape(1, L).astype(np.float32)
    return (output, h_new, c_new, attn_w), res


# revision 16
# speedup vs baseline: 1.7490x; 1.0424x over previous
"""Trainium2 Bass kernel for a single-step attention LSTM decoder (DecoderRNN).

Strategy (8 NeuronCores, SPMD):
  - The dominant cost is the vocab projection out_W (50257x1024 fp32, ~206 MB):
    sharded over vocab across the 8 cores (~26 MB/core), streamed in 8 waves of
    [contraction=1024, vocab=800] fp32r tiles used as the PE's moving operand
    (h stationary) -- ~45us of PE time, fully hidden under the DMA stream.
  - LSTM weights W_ih/W_hh (2x 4096x1024) are sharded over the contraction dim:
    each core computes partial pre-activation gates [1,4096] from its 128-wide
    slice of the hidden dim; a 16 KB AllReduce(add) produces full gates on all
    cores.
  - The a2d projection is likewise contraction-sharded for the LSTM input: each
    core only computes its own 128-slice of the a2d output (relu'd), which is
    exactly the slice its W_ih partial product needs.  No extra collective.
  - Attention (tiny) is computed replicated on every core.
  - log_softmax over the vocab: per-wave (max, sum-exp) stats on partition 0,
    combined locally, then a 64 B AllGather exchanges the 8 core-local stat
    pairs; each core computes the global logsumexp and normalizes its shard.
  - Embedding row gather + weight re-layout (transposes) happen on host inside
    kernel(); the device only streams what the math needs.
"""

import numpy as np

import concourse.bacc as bacc
import concourse.mybir as mybir
from concourse import tile
from concourse.bass_utils import run_bass_kernel_spmd

F32 = mybir.dt.float32
F32R = mybir.dt.float32r
AF = mybir.ActivationFunctionType
AX = mybir.AxisListType
N_CORES = 8
V, H, E, L = 50257, 1024, 512, 128
KC_A = (E + H) // 128          # 12 contraction chunks for attention / a2d
NCH = (V + 127) // 128         # 393 global vocab chunks
SLOTS = (NCH + N_CORES - 1) // N_CORES  # 50 vocab chunks per core
VCORE = SLOTS * 128            # 6400 vocab columns per core
NW, WM = 10, 640               # weight stream: 10 waves x 640 vocab cols
NEG = -1e30

_CACHE = {}


def _build():
    """Build + compile the SPMD Bass program (same on all cores)."""
    nc = bacc.Bacc(None, num_devices=N_CORES)

    din = {}
    for name, shape in [
        ("attn_b", [1, 128]),
        ("a2d_b", [128, 1]),            # per-core slice
        ("b_gates", [128, 32]),         # gate-interleaved bias, natural [128,32]
        ("c0_rm", [128, 8]),            # c0 row-major-p layout
        ("b_row", [1, VCORE]),          # out_b shard (padded with -1e30)
    ]:
        din[name] = nc.dram_tensor(name, shape, F32, kind="ExternalInput")
    for name, shape in [
        ("attn_in", [128, KC_A]),       # [embed;h0] column-chunk layout
        ("attn_wt", [128, KC_A, 128]),  # attn_W.T re-laid [p, kc, l]
        ("enc", [L, H]),                # natural layout
        ("a2d_wt", [128, KC_A, 128]),   # per-core slice of a2d_W.T
        ("wih_t", [128, 4 * H]),        # per-core k-slice, gate-interleaved cols
        ("whh_t", [128, 4 * H]),
        ("h0_col", [128, 1]),           # per-core k-slice of h0
        ("w_out", [NW, 128, 8, WM]),    # out weight stream tiles
    ]:
        din[name] = nc.dram_tensor(name, shape, F32R, kind="ExternalInput")

    z_out = nc.dram_tensor("z_out", [1, VCORE], F32, kind="ExternalOutput")
    h_out = nc.dram_tensor("h_out", [128, 8], F32, kind="ExternalOutput")
    c_out = nc.dram_tensor("c_out", [128, 8], F32, kind="ExternalOutput")
    aw_out = nc.dram_tensor("aw_out", [1, 128], F32, kind="ExternalOutput")

    rg = [list(range(N_CORES))]

    with tile.TileContext(nc) as tc:
        with (
            tc.tile_pool(name="const", bufs=1) as cp,
            tc.tile_pool(name="work", bufs=1) as wp,
            tc.tile_pool(name="wave", bufs=7) as wvp,
            tc.tile_pool(name="bias", bufs=2) as bp,
            tc.tile_pool(name="dram", bufs=1, space="DRAM") as dp,
        ):
            # ---- front weight loads (scalar HWDGE ring, before the big stream)
            attn_wt_sb = wvp.tile([128, KC_A, 128], F32R, tag="stream")
            nc.scalar.dma_start(attn_wt_sb[:], din["attn_wt"][:])
            a2d_wt_sb = wvp.tile([128, KC_A, 128], F32R, tag="stream")
            nc.scalar.dma_start(a2d_wt_sb[:], din["a2d_wt"][:])
            enc_sb = wvp.tile([L, H], F32R, tag="stream")
            nc.scalar.dma_start(enc_sb[:], din["enc"][:])
            wih_sb = wvp.tile([128, 4 * H], F32R, tag="stream")
            nc.scalar.dma_start(wih_sb[:], din["wih_t"][:])
            whh_sb = wvp.tile([128, 4 * H], F32R, tag="stream")
            nc.scalar.dma_start(whh_sb[:], din["whh_t"][:])

            # ---- small loads (sync ring)
            attn_in_sb = cp.tile([128, KC_A], F32R, tag="attn_in")
            nc.sync.dma_start(attn_in_sb[:], din["attn_in"][:])
            attn_b_sb = cp.tile([1, 128], F32, tag="attn_b")
            nc.sync.dma_start(attn_b_sb[:], din["attn_b"][:])
            a2d_b_sb = cp.tile([128, 1], F32, tag="a2d_b")
            nc.sync.dma_start(a2d_b_sb[:], din["a2d_b"][:])
            h0c_sb = cp.tile([128, 1], F32R, tag="h0c")
            nc.sync.dma_start(h0c_sb[:], din["h0_col"][:])
            bg_sb = cp.tile([128, 32], F32, tag="bg")
            nc.sync.dma_start(bg_sb[:], din["b_gates"][:])
            c0_sb = cp.tile([128, 8], F32, tag="c0")
            nc.sync.dma_start(c0_sb[:], din["c0_rm"][:])

            ones11 = cp.tile([1, 1], F32, tag="ones11")
            nc.vector.memset(ones11[:], 1.0)
            lnwarm = wp.tile([1, 1], F32, tag="lnwarm")
            nc.scalar.activation(lnwarm[:], ones11[:], AF.Ln)

            with tc.tile_pool(name="psmall", bufs=2, space="PSUM") as pps:
                # ================= attention (replicated) =================
                ps_a = pps.tile([1, 128], F32, tag="small")
                for kc in range(KC_A):
                    nc.tensor.matmul(
                        ps_a[:], attn_in_sb[:, kc : kc + 1], attn_wt_sb[:, kc, :],
                        start=(kc == 0), stop=(kc == KC_A - 1),
                    )
                za = wp.tile([1, 128], F32, tag="za")
                nc.vector.tensor_add(za[:], ps_a[:], attn_b_sb[:])
                # softmax over the 128 logits (all on partition 0)
                mneg = wp.tile([1, 1], F32, tag="mneg")
                nc.vector.reduce_max(mneg[:], za[:], axis=AX.X, negate=True)
                ew = wp.tile([1, 128], F32, tag="ew")
                ssum = wp.tile([1, 1], F32, tag="ssum")
                nc.scalar.activation(ew[:], za[:], AF.Exp,
                                     bias=mneg[:], accum_out=ssum[:])
                rsum = wp.tile([1, 1], F32, tag="rsum")
                nc.vector.reciprocal(rsum[:], ssum[:])
                w_row = wp.tile([1, 128], F32, tag="w_row")
                nc.vector.tensor_scalar_mul(w_row[:], ew[:], rsum[:])
                nc.sync.dma_start(aw_out[:], w_row[:])

                # transpose attn weights to a column, duplicated to 2 cols so
                # downstream fp32r matmuls have an even moving free dim
                ps_w = pps.tile([128, 1], F32, tag="small")
                nc.tensor.matmul(ps_w[:], w_row[:], ones11[:], start=True, stop=True)
                wcol2 = wp.tile([128, 2], F32R, tag="wcol2")
                nc.vector.tensor_copy(wcol2[:, 0:1], ps_w[:])
                nc.vector.tensor_copy(wcol2[:, 1:2], ps_w[:])

                # attn_out = attn_weight @ enc_output (fp32r, J=2 duplicated)
                ps_c = pps.tile([128, 16], F32, tag="small")
                for j in range(8):
                    nc.tensor.matmul(
                        ps_c[:, 2 * j : 2 * j + 2],
                        enc_sb[:, j * 128 : (j + 1) * 128],
                        wcol2[:], start=True, stop=True,
                    )
                # dec duplicated to column pairs for the J=2 a2d matmuls
                dec2 = wp.tile([128, 2 * KC_A], F32R, tag="dec2")
                nc.vector.tensor_copy(dec2[:, 0:8:2], attn_in_sb[:, 0:4])
                nc.vector.tensor_copy(dec2[:, 1:8:2], attn_in_sb[:, 0:4])
                nc.vector.tensor_copy(dec2[:, 8::2], ps_c[:, 0::2])
                nc.vector.tensor_copy(dec2[:, 9::2], ps_c[:, 0::2])

                # ===== a2d: only this core's 128-slice of the output =====
                ps_x = pps.tile([128, 2], F32, tag="small")
                for kc in range(KC_A):
                    nc.tensor.matmul(
                        ps_x[:], a2d_wt_sb[:, kc, :], dec2[:, 2 * kc : 2 * kc + 2],
                        start=(kc == 0), stop=(kc == KC_A - 1),
                    )
                x_col = wp.tile([128, 1], F32R, tag="x_col")
                nc.scalar.activation(x_col[:], ps_x[:, 0:1], AF.Relu, bias=a2d_b_sb[:])

            # ===== LSTM gates: partial [1, 4096], AllGather + local sum =====
            with tc.tile_pool(name="pgate", bufs=1, space="PSUM") as pg:
                g_dram = dp.tile([1, 4 * H], F32)
                g_part = wp.tile([1, 4 * H], F32, tag="g_part")
                ps_g = pg.tile([1, 4 * H], F32, tag="gates")
                for b in range(8):
                    lo = b * 512
                    nc.tensor.matmul(
                        ps_g[:, lo : lo + 512], h0c_sb[:],
                        whh_sb[:, lo : lo + 512], start=True, stop=False,
                    )
                for b in range(8):
                    lo = b * 512
                    nc.tensor.matmul(
                        ps_g[:, lo : lo + 512], x_col[:],
                        wih_sb[:, lo : lo + 512], start=False, stop=True,
                    )
                # PSUM can't be DMA'd; copy out on two engines in parallel
                nc.vector.tensor_copy(g_part[:, 0:2048], ps_g[:, 0:2048])
                nc.scalar.activation(g_part[:, 2048:4096], ps_g[:, 2048:4096],
                                     AF.Copy)
                nc.sync.dma_start(g_dram[:], g_part[:])
                g_ag = dp.tile([N_CORES, 4 * H], F32)
                nc.gpsimd.collective_compute(
                    "AllGather", mybir.AluOpType.bypass, replica_groups=rg,
                    ins=[g_dram.opt()], outs=[g_ag.opt()],
                )

                # gather partials as [p, rank, 32] and sum over ranks
                g8_sb = wp.tile([128, N_CORES, 32], F32, tag="g8_sb")
                nc.sync.dma_start(
                    g8_sb[:], g_ag[:].rearrange("r (p j) -> p r j", p=128))
                g_sum = wp.tile([128, 32], F32, tag="g_sum")
                nc.vector.reduce_sum(
                    g_sum[:], g8_sb[:].rearrange("p r j -> p j r"), axis=AX.X)
                t1 = wp.tile([128, 32], F32, tag="t1")
                nc.vector.tensor_add(t1[:], g_sum[:], bg_sb[:])
                t1v = t1[:].rearrange("p (a b) -> p a b", b=4)

                # gate types interleaved host-side as (i, f, o, g): one
                # sigmoid op covers i/f/o, one tanh covers g
                sg3 = wp.tile([128, 24], F32, tag="sg3")
                v3 = sg3[:].rearrange("p (a b) -> p a b", b=3)
                nc.scalar.activation(v3, t1v[:, :, 0:3], AF.Sigmoid)
                tanh_g = wp.tile([128, 8], F32, tag="tanh_g")
                nc.scalar.activation(tanh_g[:], t1v[:, :, 3], AF.Tanh)

                c_new = wp.tile([128, 8], F32, tag="c_new")
                tmp = wp.tile([128, 8], F32, tag="tmp")
                nc.vector.tensor_mul(tmp[:], v3[:, :, 1], c0_sb[:])
                nc.vector.tensor_mul(c_new[:], v3[:, :, 0], tanh_g[:])
                nc.vector.tensor_add(c_new[:], c_new[:], tmp[:])
                tanh_c = wp.tile([128, 8], F32, tag="tanh_c")
                nc.scalar.activation(tanh_c[:], c_new[:], AF.Tanh)
                h_r = wp.tile([128, 8], F32R, tag="h_r")
                nc.vector.tensor_mul(h_r[:], v3[:, :, 2], tanh_c[:])
                nc.sync.dma_start(h_out[:], h_r[:].bitcast(F32))
                nc.sync.dma_start(c_out[:], c_new[:])

            # ========== vocab projection: 8 waves of [1024 x 800] ==========
            z_row = wp.tile([1, VCORE], F32, tag="z_row")
            negm_all = wp.tile([1, NW], F32, tag="negm_all")
            s_all = wp.tile([1, NW], F32, tag="s_all")
            with tc.tile_pool(name="pz", bufs=2, space="PSUM") as zp:
                for w in range(NW):
                    wv = wvp.tile([128, 8, WM], F32R, tag="stream")
                    nc.scalar.dma_start(wv[:], din["w_out"][w])
                    bw = bp.tile([1, WM], F32, tag="bw")
                    nc.sync.dma_start(bw[:], din["b_row"][:, w * WM : (w + 1) * WM])
                    ps = zp.tile([1, WM], F32, tag="zps")
                    for lo, n in ((0, 512), (512, WM - 512)):
                        for kc in range(8):
                            nc.tensor.matmul(
                                ps[:, lo : lo + n], h_r[:, kc : kc + 1],
                                wv[:, kc, lo : lo + n],
                                start=(kc == 0), stop=(kc == 7),
                            )
                    seg = z_row[:, w * WM : (w + 1) * WM]
                    nc.vector.tensor_add(seg, ps[:], bw[:])
                    nc.vector.reduce_max(
                        negm_all[:, w : w + 1], seg, axis=AX.X, negate=True)
                    e_scr = wp.tile([1, WM], F32, tag="e_scr")
                    nc.scalar.activation(
                        e_scr[:], seg, AF.Exp,
                        bias=negm_all[:, w : w + 1],
                        accum_out=s_all[:, w : w + 1],
                    )

                # local stats:  m_loc = max_w m_w,  s_loc = sum_w s_w*e^(m_w-m_loc)
                negm_loc = wp.tile([1, 1], F32, tag="negm_loc")
                nc.vector.tensor_reduce(
                    negm_loc[:], negm_all[:], axis=AX.X, op=mybir.AluOpType.min)
                terms = wp.tile([1, NW], F32, tag="terms")
                nc.scalar.activation(terms[:], negm_all[:], AF.Exp,
                                     bias=negm_loc[:], scale=-1.0)
                nc.vector.tensor_mul(terms[:], terms[:], s_all[:])
                s_loc = wp.tile([1, 1], F32, tag="s_loc")
                nc.vector.reduce_sum(s_loc[:], terms[:], axis=AX.X)
                stats = wp.tile([1, 2], F32, tag="stats")
                nc.vector.tensor_scalar_mul(stats[:, 0:1], negm_loc[:], -1.0)
                nc.vector.tensor_copy(stats[:, 1:2], s_loc[:])

                st_in = dp.tile([1, 2], F32)
                st_all = dp.tile([N_CORES, 2], F32)
                nc.sync.dma_start(st_in[:], stats[:])
                nc.gpsimd.collective_compute(
                    "AllGather", mybir.AluOpType.bypass, replica_groups=rg,
                    ins=[st_in.opt()], outs=[st_all.opt()],
                )
                st_sb = wp.tile([1, N_CORES, 2], F32, tag="st_sb")
                nc.sync.dma_start(st_sb[:], st_all[:])

                mg = wp.tile([1, 1], F32, tag="mg")
                nc.vector.reduce_max(mg[:], st_sb[:, :, 0], axis=AX.X)
                negmg = wp.tile([1, 1], F32, tag="negmg")
                nc.vector.tensor_scalar_mul(negmg[:], mg[:], -1.0)
                terms2 = wp.tile([1, N_CORES], F32, tag="terms2")
                nc.scalar.activation(terms2[:], st_sb[:, :, 0], AF.Exp,
                                     bias=negmg[:])
                nc.vector.tensor_mul(terms2[:], terms2[:], st_sb[:, :, 1])
                stot = wp.tile([1, 1], F32, tag="stot")
                nc.vector.reduce_sum(stot[:], terms2[:], axis=AX.X)
                lnst = wp.tile([1, 1], F32, tag="lnst")
                nc.scalar.activation(lnst[:], stot[:], AF.Ln)
                neglse = wp.tile([1, 1], F32, tag="neglse")
                nc.vector.tensor_sub(neglse[:], negmg[:], lnst[:])

                # z -= lse, split across DVE and ACT, store each half as done
                half = 3648  # DVE is a bit faster; give it the larger share
                nc.vector.tensor_scalar_add(
                    z_row[:, 0:half], z_row[:, 0:half], neglse[:])
                nc.sync.dma_start(z_out[:, 0:half], z_row[:, 0:half])
                nc.scalar.activation(
                    z_row[:, half:VCORE], z_row[:, half:VCORE], AF.Identity,
                    bias=neglse[:])
                nc.sync.dma_start(z_out[:, half:VCORE], z_row[:, half:VCORE])

    nc.compile()
    return nc


def _prep_inputs(input_tok, h0, c0, enc_output, emb_table, attn_W, attn_b,
                 a2d_W, a2d_b, W_ih, W_hh, b_ih, b_hh, out_W, out_b):
    """Host-side sharding / re-layout. Returns per-core input maps + chunk map."""
    f32 = np.float32
    tok = int(np.asarray(input_tok).reshape(-1)[0])
    embed = np.asarray(emb_table[tok], dtype=f32).reshape(E)
    h0v = np.asarray(h0, dtype=f32).reshape(H)
    c0v = np.asarray(c0, dtype=f32).reshape(H)
    enc = np.ascontiguousarray(np.asarray(enc_output, dtype=f32))

    attn_in = np.concatenate([embed, h0v]).reshape(KC_A, 128).T.copy()  # [128,12]
    attn_wt = (
        np.asarray(attn_W, dtype=f32).T.reshape(KC_A, 128, 128).transpose(1, 0, 2).copy()
    )  # [128, kc, l]
    attn_b_r = np.asarray(attn_b, dtype=f32).reshape(1, 128).copy()

    a2d_wt_full = np.asarray(a2d_W, dtype=f32).T  # (1536, 1024)
    GPERM = [0, 1, 3, 2]  # torch (i,f,g,o) -> interleave order (i,f,o,g)
    b_gate = (np.asarray(b_ih, dtype=f32) + np.asarray(b_hh, dtype=f32))
    b_gate_il = b_gate.reshape(4, H)[GPERM].T.reshape(128, 32).copy()
    c0_rm = c0v.reshape(128, 8).copy()

    # gate-interleaved, transposed LSTM weights (1024, 4096)
    wih_il_t = np.asarray(W_ih, dtype=f32).reshape(4, H, H)[GPERM].transpose(2, 1, 0).reshape(H, 4 * H)
    whh_il_t = np.asarray(W_hh, dtype=f32).reshape(4, H, H)[GPERM].transpose(2, 1, 0).reshape(H, 4 * H)

    out_Wf = np.asarray(out_W, dtype=f32)
    out_bf = np.asarray(out_b, dtype=f32)

    chunk_map = [[] for _ in range(N_CORES)]
    for j in range(NCH):
        chunk_map[j % N_CORES].append(j)
    for c in range(N_CORES):
        while len(chunk_map[c]) < SLOTS:
            chunk_map[c].append(-1)  # zero-pad slot

    in_maps = []
    for c in range(N_CORES):
        chunks = chunk_map[c]
        arr = np.zeros((SLOTS, 128, H), dtype=f32)
        b_row_c = np.full(VCORE, NEG, dtype=f32)
        for s, j in enumerate(chunks):
            if j < 0:
                continue
            lo, hi = j * 128, min((j + 1) * 128, V)
            n = hi - lo
            arr[s, :n] = out_Wf[lo:hi]
            b_row_c[s * 128 : s * 128 + n] = out_bf[lo:hi]
        # arr[s, i, r] -> wave tiles [w, p, kc, m] with r = p*8+kc, m = w*WM+m'
        w_out_c = np.ascontiguousarray(
            arr.reshape(VCORE, H).T          # (r, m)
            .reshape(128, 8, NW, WM)         # (p, kc, w, m')
            .transpose(2, 0, 1, 3)           # (w, p, kc, m')
        )
        a2d_wt_c = np.ascontiguousarray(
            a2d_wt_full[:, c * 128 : (c + 1) * 128]
            .reshape(KC_A, 128, 128)
            .transpose(1, 0, 2)
        )
        in_maps.append({
            "attn_in": attn_in,
            "attn_wt": attn_wt,
            "attn_b": attn_b_r,
            "enc": enc,
            "a2d_wt": a2d_wt_c,
            "a2d_b": np.asarray(a2d_b, dtype=f32)[c * 128 : (c + 1) * 128].reshape(128, 1).copy(),
            "wih_t": np.ascontiguousarray(wih_il_t[c * 128 : (c + 1) * 128]),
            "whh_t": np.ascontiguousarray(whh_il_t[c * 128 : (c + 1) * 128]),
            "h0_col": h0v[c * 128 : (c + 1) * 128].reshape(128, 1).copy(),
            "b_gates": b_gate_il,
            "c0_rm": c0_rm,
            "w_out": w_out_c,
            "b_row": b_row_c.reshape(1, VCORE),
        })
    return in_maps, chunk_map


def kernel(**inputs):
    outputs, _ = _run(inputs, trace=False)
    return outputs


def run_traced(inputs):
    """test-only entry: returns (outputs, BassKernelResults) with a HW trace."""
    return _run(inputs, trace=True)


def _run(inputs, trace):
    if "nc" not in _CACHE:
        _CACHE["nc"] = _build()
    nc = _CACHE["nc"]

    in_maps, chunk_map = _prep_inputs(**inputs)
    res = run_bass_kernel_spmd(
        nc, in_maps, core_ids=list(range(N_CORES)), trace=trace
    )
    results = res.results

    full = np.empty(NCH * 128, dtype=np.float32)
    for c in range(N_CORES):
        z_c = results[c]["z_out"].reshape(VCORE)
        for s, j in enumerate(chunk_map[c]):
            if j >= 0:
                full[j * 128 : (j + 1) * 128] = z_c[s * 128 : (s + 1) * 128]
    output = full[:V].reshape(1, V)

    h_new = results[0]["h_out"].reshape(1, 1, H).astype(np.float32)
    c_new = results[0]["c_out"].reshape(1, 1, H).astype(np.float32)
    attn_w = results[0]["aw_out"].reshape(1, L).astype(np.float32)
    return (output, h_new, c_new, attn_w), res
